# revision 1
# baseline (speedup 1.0000x reference)
"""Trainium2 Bass kernel for nn_CrossLayer (dense transformer layer), v3.

Sharding: sequence-parallel over 8 cores (2 samples x 4 token-chunks of 512).
Each core computes its 512 token rows through CA -> SA -> FFN.  k/v for
both attention blocks are computed from each core's own 512 rows and
AllGather'd across the 4 cores of its sample in 4 pipelined head-group
chunks (fp8 wire), overlapped with attention of earlier groups.

Precision split (chosen by per-site numpy error attribution):
- attention: fully fp8 e4m3 with DoubleRow matmuls (256-contract, 2x)
  for q/k/v/Wo projections and AV; scores plain-fp8 (64-contract).
  Weights pre-scaled x8; the /8s fold into norm masks (qn/8) and the
  softmax-denominator exp bias (-ln 64).
- FFN: bf16 end to end (fp8 W1/W3/h each cost ~1e-2 rel err; bf16 is
  only +82us of PE and keeps total err ~7e-3).
- exp(score/8 - 3): |score| <= 8 after q/k rms-norm so probs fit fp8
  without max subtraction; e^-3 cancels in the denominator.
- norm/rope engine split: ACT raw/Ln/Exp/rot-copy, DVE sq/v1/t1/d0/add,
  GpSimd left free so AllGather triggers fire immediately.
"""

import math
import sys
import types

import numpy as np
import ml_dtypes

B, N, DIM, HID, H, D = 2, 2048, 1024, 4096, 16, 64
TOK = 512  # tokens per core
NCORES = 8
EPS = 1e-6
THETA = 10000.0
P = 128
KO = DIM // P  # 8 contraction chunks
KOP = KO // 2  # 4 DoubleRow pair-chunks
HH = H // 2  # 8 head pairs
HC = HID // P  # 32 hidden chunks
TC = TOK // P  # 4 token chunks per core
NR = 4  # ranks per replica group
SRCN = 2048  # gathered kv tokens
SKC = SRCN // P  # 16 key chunks of 128 tokens
VW = D + 1  # v columns + ones column
HG = 4  # head groups (2 head-pairs each)
WS = 8.0  # attention weight pre-scale for fp8
LN64 = math.log(64.0)

BF = ml_dtypes.bfloat16
F8 = ml_dtypes.float8_e4m3

_cache = {}


def _lhsT_dr(W):
    """[K, M] -> [M//128, 128(K%128), K//256, 2, 128(M%128)] fp8 x8.
    Slice [mt][:, kp] is a DoubleRow lhsT [128, 2, 128]."""
    K, M = W.shape
    A = W.reshape(K // 256, 2, P, M // P, P)
    return (A.transpose(3, 2, 0, 1, 4) * WS).astype(F8).copy()


def _rhs_dr(W):
    """[K, M] -> [128, K//256, 2, M] fp8 x8 rhs-style DoubleRow moving."""
    K, M = W.shape
    A = W.reshape(K // 256, 2, P, M)
    return (A.transpose(2, 0, 1, 3) * WS).astype(F8).copy()


def _lhsT_bf(W):
    """[K, M] -> [M//128, 128(K%128), K//128, 128(M%128)] bf16 (unscaled)."""
    K, M = W.shape
    return W.reshape(K // P, P, M // P, P).transpose(2, 1, 0, 3).astype(BF).copy()


def _featmajor(x, dt):
    """[tok, dim] -> [128, dim//128, tok]."""
    n = x.shape[0]
    return x.T.reshape(DIM // P, P, n).transpose(1, 0, 2).astype(dt).copy()


def _rope_tables(pos):
    """pos [n] int32 -> cos/sin [128, n] (2 heads stacked) bf16."""
    n = pos.shape[0]
    invf = 1.0 / (THETA ** (np.arange(0, D, 2, dtype=np.float64) / D))
    ang = pos.astype(np.float64)[None, :] * invf[:, None]  # [32, n]
    c = np.cos(ang)
    s = np.sin(ang)
    c64 = np.concatenate([c, c], axis=0)  # [64, n]
    s64 = np.concatenate([s, s], axis=0)
    c128 = np.concatenate([c64, c64], axis=0).astype(BF)  # [128, n]
    s128 = np.concatenate([s64, s64], axis=0).astype(BF)
    return c128.copy(), s128.copy()


def _install_ntff_hook():
    try:
        from trn_agent_boot.trn_boot import _ntff_profile_via_ctypes
    except ImportError:
        return
    if "antenv.axon_hooks" in sys.modules:
        return
    try:
        hook = _ntff_profile_via_ctypes("/opt/axon/libaxon_pjrt.so")
    except OSError:
        return
    mod = types.ModuleType("antenv.axon_hooks")
    mod.get_axon_ntff_profile_hook = lambda: hook
    mod.set_axon_ntff_profile_hook = lambda h: None
    sys.modules["antenv.axon_hooks"] = mod
    import antenv

    antenv.axon_hooks = mod


def _split_multiwait(nc):
    """This walrus only supports one sync-wait on CTRL-encoded instructions
    (Drain/NoOp); hoist excess waits onto single-wait NoOps placed before."""
    from concourse import mybir

    n_split = 0
    for f in nc.m.functions:
        for bb in f.blocks:
            new = []
            changed = False
            for ins in bb.instructions:
                si = ins.sync_info
                if (
                    si is not None
                    and si.on_wait is not None
                    and len(si.on_wait) > 1
                ):
                    waits = list(si.on_wait)
                    keep, rest = waits[:1], waits[1:]
                    for k, w in enumerate(rest):
                        new.append(
                            mybir.InstNoOp(
                                name=f"{ins.name}-wsplit{k}",
                                engine=ins.engine,
                                sync_info=mybir.SyncInfo(
                                    on_wait=[w], on_update=[]
                                ),
                                bass_nofuse=True,
                            )
                        )
                    si.on_wait = keep
                    n_split += 1
                    changed = True
                new.append(ins)
            if changed:
                bb.instructions = new
    return n_split


def _build_bass():
    from contextlib import ExitStack

    import concourse.bass as bass
    import concourse.tile as tile
    from concourse import mybir

    f32 = mybir.dt.float32
    bf16 = mybir.dt.bfloat16
    fp8 = mybir.dt.float8e4
    AF = mybir.ActivationFunctionType
    DR = mybir.MatmulPerfMode.DoubleRow

    nc = bass.Bass(num_devices=NCORES)

    def inp(name, shape, dt=fp8):
        return nc.dram_tensor(name, shape, dt, kind="ExternalInput")

    tgtT = inp("tgtT", [P, KO, TOK], f32)
    srcT = inp("srcT", [P, KO, TOK])
    cosq = inp("cosq", [P, TOK], bf16)
    sinq = inp("sinq", [P, TOK], bf16)
    coska = inp("coska", [P, TOK], bf16)
    sinka = inp("sinka", [P, TOK], bf16)
    caWq = inp("caWq", [HH, P, KOP, 2, P])
    caWk = inp("caWk", [HH, P, KOP, 2, P])
    caWv = inp("caWv", [P, KOP, 2, DIM])
    caWo = inp("caWo", [KO, P, KOP, 2, P])
    saWq = inp("saWq", [HH, P, KOP, 2, P])
    saWk = inp("saWk", [HH, P, KOP, 2, P])
    saWv = inp("saWv", [P, KOP, 2, DIM])
    saWo = inp("saWo", [KO, P, KOP, 2, P])
    W1i = inp("W1", [HC, P, KO, P], bf16)
    W3i = inp("W3", [HC, P, KO, P], bf16)
    W2i = inp("W2", [KO, P, HC, P], bf16)
    blk2 = inp("blk2", [P, 2], bf16)  # per-head ssq lhsT (block ones)
    mq_ca = inp("mq_ca", [2, P], bf16)  # rsqrt bcast lhsT, qn/8 folded
    mk_ca = inp("mk_ca", [2, P], bf16)
    mq_sa = inp("mq_sa", [2, P], bf16)
    mk_sa = inp("mk_sa", [2, P], bf16)
    rotm = inp("rotm", [P, P], bf16)  # rotate-half (2-head block diag) lhsT
    ones_c = inp("ones_c", [P, 1], bf16)  # y-norm ssq lhsT
    ones_r128 = inp("ones_r128", [1, P], bf16)  # y-norm bcast lhsT

    outT = nc.dram_tensor("outT", [P, KO, TOK], f32, kind="ExternalOutput")

    groups = [[0, 1, 2, 3], [4, 5, 6, 7]]
    KSZ = P * 2 * TOK  # k fp8 words per rank per head-group
    VSZ = P * TC * 4 * VW  # v fp8 words per rank per head-group

    with tile.TileContext(nc) as tc:
        ctx = ExitStack()
        with ctx:
            sing = ctx.enter_context(tc.tile_pool(name="sing", bufs=1))
            big = ctx.enter_context(tc.tile_pool(name="big", bufs=1))
            wpool = ctx.enter_context(tc.tile_pool(name="wpool", bufs=2))
            w13p = ctx.enter_context(tc.tile_pool(name="w13p", bufs=2))
            w2p = ctx.enter_context(tc.tile_pool(name="w2p", bufs=1))
            htp = ctx.enter_context(tc.tile_pool(name="htp", bufs=2))
            work = ctx.enter_context(tc.tile_pool(name="work", bufs=2))
            probp = ctx.enter_context(tc.tile_pool(name="probp", bufs=2))
            stat = ctx.enter_context(tc.tile_pool(name="stat", bufs=2))
            dram = ctx.enter_context(
                tc.tile_pool(name="dram", bufs=1, space="DRAM")
            )
            # PSUM: pk 2 + pp 2 + ps 2 + px0/px1 2 = 8 banks exactly
            pkp = ctx.enter_context(tc.tile_pool(name="pkp", bufs=2, space="PSUM"))
            pp = ctx.enter_context(tc.tile_pool(name="pp", bufs=2, space="PSUM"))
            psp = ctx.enter_context(tc.tile_pool(name="psp", bufs=1, space="PSUM"))
            pxp = ctx.enter_context(tc.tile_pool(name="pxp", bufs=1, space="PSUM"))

            # ---- resident tiles
            resid = sing.tile([P, KO, TOK], f32)
            nc.sync.dma_start(resid[:], tgtT[:])
            srcT_sb = sing.tile([P, KO, TOK], fp8, name="srcT_sb")
            nc.sync.dma_start(srcT_sb[:], srcT[:])
            cosq_sb = sing.tile([P, TOK], bf16)
            nc.sync.dma_start(cosq_sb[:], cosq[:])
            sinq_sb = sing.tile([P, TOK], bf16)
            nc.sync.dma_start(sinq_sb[:], sinq[:])
            coska_sb = sing.tile([P, TOK], bf16)
            nc.sync.dma_start(coska_sb[:], coska[:])
            sinka_sb = sing.tile([P, TOK], bf16)
            nc.sync.dma_start(sinka_sb[:], sinka[:])
            blk2_sb = sing.tile([P, 2], bf16)
            nc.sync.dma_start(blk2_sb[:], blk2[:])
            masks_sb = {}
            for name, t in (
                ("mq_ca", mq_ca),
                ("mk_ca", mk_ca),
                ("mq_sa", mq_sa),
                ("mk_sa", mk_sa),
            ):
                m = sing.tile([2, P], bf16, name=name)
                nc.sync.dma_start(m[:], t[:])
                masks_sb[name] = m
            rotm_sb = sing.tile([P, P], bf16)
            nc.sync.dma_start(rotm_sb[:], rotm[:])
            ones_c_sb = sing.tile([P, 1], bf16)
            nc.sync.dma_start(ones_c_sb[:], ones_c[:])
            ones_r128_sb = sing.tile([1, P], bf16)
            nc.sync.dma_start(ones_r128_sb[:], ones_r128[:])
            eps_sb = sing.tile([2, 1], f32)
            nc.vector.memset(eps_sb[:], float(EPS))
            bm3_sb = sing.tile([P, 1], f32)
            nc.vector.memset(bm3_sb[:], -3.0)
            bln64_sb = sing.tile([P, 1], f32)
            nc.vector.memset(bln64_sb[:], -LN64)

            yT = sing.tile([P, KO, TOK], fp8, name="yT")
            yF = sing.tile([P, KO, TOK], bf16, name="yF")
            q_sb = sing.tile([P, HH, TOK], fp8, name="q_sb")
            xT = sing.tile([P, HH, TOK], fp8, name="xT")
            k_mine = sing.tile([P, HH, TOK], fp8, name="k_mine")
            v_mine = sing.tile([P, TC, H, VW], fp8, name="v_mine")
            nc.vector.memset(v_mine[:, :, :, D : D + 1], 1.0)
            k_full = big.tile([P, HH, SRCN], fp8, tag="k_full", name="k_full")
            v_full = big.tile([P, SKC, H, VW], fp8, tag="v_full", name="v_full")
            nc.vector.memset(v_full[:, :, :, D : D + 1], 1.0)

            def norm_rope_one(pk, mask_sb, cos_sb, sin_sb, dst):
                """pk PSUM [128(2 heads), T] f32 at 8x scale -> dst fp8:
                rms-normed, qn-scaled, roped.  ACT: raw/Ln/Exp/rot-copy;
                DVE: sq/v1/t1/d0/add; PE: 3 small matmuls; GpSimd: free."""
                T = pk.shape[-1]
                raw = stat.tile([P, T], f32, tag="raw", name="raw")
                nc.scalar.copy(raw[:], pk[:])
                sq = work.tile([P, T], bf16, tag="sq", name="sq")
                nc.vector.tensor_mul(sq[:], raw[:], raw[:])
                ssq = pp.tile([2, T], f32, tag="pp", name="ssq")
                nc.tensor.matmul(ssq[:], blk2_sb[:], sq[:], start=True, stop=True)
                # rsqrt(mean+eps) = exp(-0.5*ln(mean+eps)); Ln/Exp share one
                # ACT table set with the attention exps.  1/(64*D) unscales
                # the x8 weight prescale (squared).
                lnt = stat.tile([2, T], f32, tag="lnt", name="lnt")
                nc.scalar.activation(
                    lnt[:], ssq[:], AF.Ln, bias=eps_sb[:], scale=1.0 / (64 * D)
                )
                rs = stat.tile([2, T], bf16, tag="rs", name="rs")
                nc.scalar.activation(rs[:], lnt[:], AF.Exp, scale=-0.5)
                bc = pp.tile([P, T], f32, tag="pp", name="bc")
                nc.tensor.matmul(bc[:], mask_sb[:], rs[:], start=True, stop=True)
                v1 = stat.tile([P, T], bf16, tag="v1", name="v1")
                nc.vector.tensor_mul(v1[:], raw[:], bc[:])
                rot_ps = pp.tile([P, T], f32, tag="pp", name="rot_ps")
                nc.tensor.matmul(
                    rot_ps[:], rotm_sb[:], v1[:], start=True, stop=True
                )
                rot = stat.tile([P, T], bf16, tag="rot", name="rot")
                nc.scalar.copy(rot[:], rot_ps[:])
                t1 = work.tile([P, T], bf16, tag="t1", name="t1")
                nc.vector.tensor_mul(t1[:], v1[:], cos_sb)
                d0 = work.tile([P, T], bf16, tag="d0", name="d0")
                nc.vector.tensor_mul(d0[:], rot[:], sin_sb)
                nc.vector.tensor_add(dst, t1[:], d0[:])

            def rmsnorm_feat(dst):
                """resid f32 -> dst (fp8 or bf16): resid * rsqrt(mean sq)."""
                ssq = pp.tile([1, TOK], f32, tag="pp", name="yssq")
                for c in range(KO):
                    sq = work.tile([P, TOK], bf16, tag="sq", name="ynsq")
                    nc.vector.tensor_mul(sq[:], resid[:, c], resid[:, c])
                    nc.tensor.matmul(
                        ssq[:],
                        ones_c_sb[:],
                        sq[:],
                        start=(c == 0),
                        stop=(c == KO - 1),
                    )
                lnt = stat.tile([1, TOK], f32, tag="lnt", name="ylnt")
                nc.scalar.activation(
                    lnt[:], ssq[:], AF.Ln, bias=eps_sb[:1], scale=1.0 / DIM
                )
                rs = stat.tile([1, TOK], bf16, tag="rs", name="yrs")
                nc.scalar.activation(rs[:], lnt[:], AF.Exp, scale=-0.5)
                bc = pp.tile([P, TOK], f32, tag="pp", name="ybc")
                nc.tensor.matmul(
                    bc[:], ones_r128_sb[:], rs[:], start=True, stop=True
                )
                for c in range(KO):
                    nc.vector.tensor_mul(dst[:, c], resid[:, c], bc[:])

            def kv_group_and_ag(hg, ysrc, Wk_t, Wv_t, mask, cos_sb, sin_sb,
                                kv_in, kv_out):
                """k (2 head-pairs) + v (4 heads) from my 512 rows -> AG."""
                wk = wpool.tile([P, 2, KOP, 2, P], fp8, tag="wk2", name="wk")
                nc.sync.dma_start(
                    wk[:],
                    Wk_t[2 * hg : 2 * hg + 2].rearrange("h p a b m -> p h a b m"),
                )
                wv = wpool.tile([P, KOP, 2, 256], fp8, tag="wv", name="wv")
                nc.sync.dma_start(
                    wv[:], Wv_t[:, :, :, hg * 256 : (hg + 1) * 256]
                )
                for j in range(2):
                    pk = pkp.tile([P, TOK], f32, tag="pk", name="pkk")
                    for kp in range(KOP):
                        nc.tensor.matmul(
                            pk[:],
                            wk[:, j, kp],
                            ysrc[:, 2 * kp : 2 * kp + 2, :],
                            start=(kp == 0),
                            stop=(kp == KOP - 1),
                            perf_mode=DR,
                        )
                    # raw copy frees the psum slot right away
                    norm_rope_one(
                        pk, mask, cos_sb[:], sin_sb[:],
                        k_mine[:, 2 * hg + j],
                    )
                for t in range(TC):
                    pvt = pp.tile([P, TOK], f32, tag="pp", name="pv")
                    pv = pvt[:, 0:256]
                    for kp in range(KOP):
                        nc.tensor.matmul(
                            pv,
                            ysrc[:, 2 * kp : 2 * kp + 2, t * P : (t + 1) * P],
                            wv[:, kp],
                            start=(kp == 0),
                            stop=(kp == KOP - 1),
                            perf_mode=DR,
                        )
                    nc.vector.tensor_copy(
                        v_mine[:, t, 4 * hg : 4 * hg + 4, 0:D],
                        pv.rearrange("p (h d) -> p h d", d=D),
                    )
                nc.sync.dma_start(
                    kv_in[:KSZ].rearrange("(p j t) -> p j t", p=P, j=2, t=TOK),
                    k_mine[:, 2 * hg : 2 * hg + 2, :],
                )
                nc.sync.dma_start(
                    kv_in[KSZ:].rearrange(
                        "(p a b c) -> p a b c", p=P, a=TC, b=4, c=VW
                    ),
                    v_mine[:, :, 4 * hg : 4 * hg + 4, :],
                )
                nc.gpsimd.collective_compute(
                    "AllGather",
                    mybir.AluOpType.bypass,
                    replica_groups=groups,
                    ins=[kv_in.opt()],
                    outs=[kv_out.opt()],
                )

            def scatter_group(hg, kv_out):
                for r in range(NR):
                    nc.sync.dma_start(
                        k_full[:, 2 * hg : 2 * hg + 2, r * TOK : (r + 1) * TOK],
                        kv_out[r, :KSZ].rearrange(
                            "(p j t) -> p j t", p=P, j=2, t=TOK
                        ),
                    )
                    nc.sync.dma_start(
                        v_full[:, r * TC : (r + 1) * TC, 4 * hg : 4 * hg + 4, :],
                        kv_out[r, KSZ:].rearrange(
                            "(p a b c) -> p a b c", p=P, a=TC, b=4, c=VW
                        ),
                    )

            def proj_q(Wt, mask):
                """y -> q (all 8 head-pairs), normed+roped into q_sb."""
                for g in range(HG):
                    wq = wpool.tile([P, 2, KOP, 2, P], fp8, tag="wk2", name="wq")
                    nc.sync.dma_start(
                        wq[:],
                        Wt[2 * g : 2 * g + 2].rearrange("h p a b m -> p h a b m"),
                    )
                    pks = []
                    for j in range(2):
                        pk = pkp.tile([P, TOK], f32, tag="pk", name="pq")
                        for kp in range(KOP):
                            nc.tensor.matmul(
                                pk[:],
                                wq[:, j, kp],
                                yT[:, 2 * kp : 2 * kp + 2, :],
                                start=(kp == 0),
                                stop=(kp == KOP - 1),
                                perf_mode=DR,
                            )
                        pks.append(pk)
                    for j in range(2):
                        norm_rope_one(
                            pks[j], mask, cosq_sb[:], sinq_sb[:],
                            q_sb[:, 2 * g + j],
                        )

            def attention_group(hg, kdb):
                """scores+softmax+AV for head-pairs 2hg,2hg+1; fills
                xT[:, 2hg:2hg+2] with x_norm/8 (fp8)."""
                xraw = stat.tile([P, 2, TOK], bf16, tag="xraw", name="xraw")
                dens4 = work.tile([P, 4, TOK], bf16, tag="dens", name="dens4")
                for j in range(2):
                    hh = 2 * hg + j
                    px = [
                        pxp.tile([VW, TOK], f32, tag=f"px{i}", name=f"px{i}")
                        for i in range(2)
                    ]
                    for kc in range(SKC):
                        ps = psp.tile([P, 2, TOK], f32, tag="ps", name="ps")
                        for i in range(2):
                            off = i * D
                            nc.tensor.matmul(
                                ps[:, i],
                                k_full[
                                    off : off + D, hh, kc * P : (kc + 1) * P
                                ],
                                q_sb[off : off + D, hh],
                                start=True,
                                stop=True,
                            )
                        if kc % 2 == 0:
                            prob = probp.tile(
                                [P, 2, 2, TOK], fp8, tag="prob", name="prob"
                            )
                        nc.scalar.activation(
                            prob[:, kc % 2],
                            ps[:],
                            AF.Exp,
                            scale=1.0 / math.sqrt(D),
                            bias=bm3_sb[:],
                        )
                        if kc % 2 == 1:
                            j2 = kc - 1
                            for i in range(2):
                                nc.tensor.matmul(
                                    px[i][:],
                                    v_full[:, j2 : j2 + 2, hh * 2 + i, :],
                                    prob[:, :, i, :],
                                    start=(kc == 1),
                                    stop=(kc == SKC - 1),
                                    perf_mode=DR,
                                )
                    for i in range(2):
                        # denom row rides on partition 64 (ones column of v)
                        nc.vector.tensor_copy(
                            dens4[D : D + 1, 2 * j + i], px[i][D : D + 1]
                        )
                        # 64-channel copy may retarget the other half-window
                        nc.vector.tensor_copy(
                            xraw[i * D : (i + 1) * D, j], px[i][0:D]
                        )
                # reciprocals: 1/(64*den); the 64 unscales v and Wo x8 each,
                # making xT = x_norm/8 which Wo's x8 restores
                nc.sync.dma_start(
                    kdb[: 4 * TOK].rearrange("(o f t) -> o f t", o=1, f=4),
                    dens4[D : D + 1],
                )
                d4 = stat.tile([4, TOK], bf16, tag="d4", name="d4")
                nc.sync.dma_start(
                    d4[:], kdb[: 4 * TOK].rearrange("(f t) -> f t", f=4)
                )
                nc.scalar.activation(d4[:], d4[:], AF.Ln)
                rec4 = stat.tile([4, TOK], bf16, tag="rec4", name="rec4")
                nc.scalar.activation(
                    rec4[:], d4[:], AF.Exp, scale=-1.0, bias=bln64_sb[:4]
                )
                nc.sync.dma_start(
                    kdb[4 * TOK :].rearrange("(f t) -> f t", f=4), rec4[:]
                )
                rec_bc = work.tile([P, 2, TOK], bf16, tag="recbc", name="rec_bc")
                for i in range(2):
                    src = bass.AP(
                        tensor=kdb.tensor,
                        offset=kdb.offset + 4 * TOK + i * TOK,
                        ap=[[0, D], [2 * TOK, 2], [1, TOK]],
                    )
                    nc.sync.dma_start(rec_bc[i * D : (i + 1) * D], src)
                for j in range(2):
                    nc.vector.tensor_mul(
                        xT[:, 2 * hg + j], xraw[:, j], rec_bc[:, j]
                    )

            def wo_group(hg, Wo_t):
                """Wo partial for head-pair chunk hg, accumulated into resid."""
                wo = wpool.tile([P, KO, 2, P], fp8, tag="wo", name="wo")
                nc.sync.dma_start(
                    wo[:], Wo_t[:, :, hg].rearrange("o p b m -> p o b m")
                )
                for oc in range(KO):
                    po = pkp.tile([P, TOK], f32, tag="pk", name="po")
                    nc.tensor.matmul(
                        po[:],
                        wo[:, oc],
                        xT[:, 2 * hg : 2 * hg + 2, :],
                        start=True,
                        stop=True,
                        perf_mode=DR,
                    )
                    nc.vector.tensor_add(resid[:, oc], resid[:, oc], po[:])

            # ================= cross-attention =================
            # CA kv depends only on src: fire projections + AllGathers first
            # so they overlap the collectives entry barrier.
            kvi_ca = [
                dram.tile([KSZ + VSZ], fp8, tag=f"kvica{g}", name=f"kvica{g}")
                for g in range(HG)
            ]
            kvo_ca = [
                dram.tile([NR, KSZ + VSZ], fp8, tag=f"kvoca{g}", name=f"kvoca{g}")
                for g in range(HG)
            ]
            kdbs = [
                dram.tile([8 * TOK], bf16, tag=f"kdb{g}", name=f"kdb{g}")
                for g in range(HG)
            ]
            for hg in range(HG):
                kv_group_and_ag(
                    hg, srcT_sb, caWk, caWv, masks_sb["mk_ca"],
                    coska_sb, sinka_sb, kvi_ca[hg], kvo_ca[hg],
                )
            rmsnorm_feat(yT)
            proj_q(caWq, masks_sb["mq_ca"])
            for hg in range(HG):
                scatter_group(hg, kvo_ca[hg])
                attention_group(hg, kdbs[hg])
                if hg >= 1:
                    wo_group(hg - 1, caWo)
            wo_group(HG - 1, caWo)

            # ================= self-attention =================
            rmsnorm_feat(yT)
            kvi_sa = [
                dram.tile([KSZ + VSZ], fp8, tag=f"kvisa{g}", name=f"kvisa{g}")
                for g in range(HG)
            ]
            kvo_sa = [
                dram.tile([NR, KSZ + VSZ], fp8, tag=f"kvosa{g}", name=f"kvosa{g}")
                for g in range(HG)
            ]
            kdbs2 = [
                dram.tile([8 * TOK], bf16, tag=f"kdc{g}", name=f"kdc{g}")
                for g in range(HG)
            ]
            for hg in range(HG):
                kv_group_and_ag(
                    hg, yT, saWk, saWv, masks_sb["mk_sa"],
                    cosq_sb, sinq_sb, kvi_sa[hg], kvo_sa[hg],
                )
                if hg == 0:
                    proj_q(saWq, masks_sb["mq_sa"])
            for hg in range(HG):
                scatter_group(hg, kvo_sa[hg])
                attention_group(hg, kdbs2[hg])
                if hg >= 1:
                    wo_group(hg - 1, saWo)
            wo_group(HG - 1, saWo)

            # ================= FFN (bf16) =================
            rmsnorm_feat(yF)
            for qtr in range(4):
                hT = htp.tile([P, 8, TOK], bf16, tag="hT", name="hT")
                for e in range(2):
                    w1 = w13p.tile([P, 4, KO, P], bf16, tag="w13", name="w1")
                    nc.sync.dma_start(
                        w1[:],
                        W1i[qtr * 8 + e * 4 : qtr * 8 + e * 4 + 4].rearrange(
                            "h p a m -> p h a m"
                        ),
                    )
                    w3 = w13p.tile([P, 4, KO, P], bf16, tag="w13", name="w3")
                    nc.sync.dma_start(
                        w3[:],
                        W3i[qtr * 8 + e * 4 : qtr * 8 + e * 4 + 4].rearrange(
                            "h p a m -> p h a m"
                        ),
                    )
                    for g in range(4):
                        p1 = pkp.tile([P, TOK], f32, tag="pk", name="p1")
                        for c in range(KO):
                            nc.tensor.matmul(
                                p1[:], w1[:, g, c], yF[:, c],
                                start=(c == 0), stop=(c == KO - 1),
                            )
                        p3 = pp.tile([P, TOK], f32, tag="pp", name="p3")
                        for c in range(KO):
                            nc.tensor.matmul(
                                p3[:], w3[:, g, c], yF[:, c],
                                start=(c == 0), stop=(c == KO - 1),
                            )
                        s1 = stat.tile([P, TOK], bf16, tag="raw", name="s1")
                        nc.scalar.activation(s1[:], p1[:], AF.Silu)
                        nc.vector.tensor_mul(hT[:, e * 4 + g], s1[:], p3[:])
                w2 = w2p.tile([P, KO, 8, P], bf16, tag="w2", name="w2")
                nc.sync.dma_start(
                    w2[:],
                    W2i[:, :, qtr * 8 : (qtr + 1) * 8].rearrange(
                        "o p a m -> p o a m"
                    ),
                )
                for oc in range(KO):
                    po = pkp.tile([P, TOK], f32, tag="pk", name="po2")
                    for g in range(8):
                        nc.tensor.matmul(
                            po[:], w2[:, oc, g], hT[:, g],
                            start=(g == 0), stop=(g == 7),
                        )
                    nc.vector.tensor_add(resid[:, oc], resid[:, oc], po[:])

            nc.sync.dma_start(outT[:], resid[:])

    _split_multiwait(nc)
    return nc


def _prep_inputs(inputs):
    """Full problem inputs -> list of 8 per-core in_maps."""
    tgt = np.asarray(inputs["tgt"], np.float32)
    src = np.asarray(inputs["src"], np.float32)
    tgt_pos = np.asarray(inputs["tgt_pos"], np.int32)
    src_pos = np.asarray(inputs["src_pos"], np.int32)

    pre_ca_w = np.asarray(inputs["pre_ca_w"], np.float32)
    pre_sa_w = np.asarray(inputs["pre_sa_w"], np.float32)
    pre_ffn_w = np.asarray(inputs["pre_ffn_w"], np.float32)

    def fold(Wname, w):
        return np.asarray(inputs[Wname], np.float32) * w[:, None]

    ca_Wq = fold("ca_Wq", pre_ca_w)
    ca_Wkv = np.asarray(inputs["ca_Wkv"], np.float32)
    ca_Wk, ca_Wv = ca_Wkv[:, :DIM], ca_Wkv[:, DIM:]
    ca_Wo = np.asarray(inputs["ca_Wo"], np.float32)
    sa_Wq = fold("sa_Wq", pre_sa_w)
    sa_Wkv = fold("sa_Wkv", pre_sa_w)
    sa_Wk, sa_Wv = sa_Wkv[:, :DIM], sa_Wkv[:, DIM:]
    sa_Wo = np.asarray(inputs["sa_Wo"], np.float32)
    W1 = fold("W1", pre_ffn_w)
    W3 = fold("W3", pre_ffn_w)
    W2 = np.asarray(inputs["W2"], np.float32)

    shared = {
        "caWq": _lhsT_dr(ca_Wq),
        "caWk": _lhsT_dr(ca_Wk),
        "caWv": _rhs_dr(ca_Wv),
        "caWo": _lhsT_dr(ca_Wo),
        "saWq": _lhsT_dr(sa_Wq),
        "saWk": _lhsT_dr(sa_Wk),
        "saWv": _rhs_dr(sa_Wv),
        "saWo": _lhsT_dr(sa_Wo),
        "W1": _lhsT_bf(W1),
        "W3": _lhsT_bf(W3),
        "W2": _lhsT_bf(W2),
    }

    blk2 = np.zeros((P, 2), BF)
    blk2[:D, 0] = 1
    blk2[D:, 1] = 1
    shared["blk2"] = blk2

    def head_mask(w):  # [2, 128] with per-head norm weight / 8
        m = np.zeros((2, P), np.float32)
        m[0, :D] = w / WS
        m[1, D:] = w / WS
        return m.astype(BF).copy()

    shared["mq_ca"] = head_mask(np.asarray(inputs["ca_qn"], np.float32))
    shared["mk_ca"] = head_mask(np.asarray(inputs["ca_kn"], np.float32))
    shared["mq_sa"] = head_mask(np.asarray(inputs["sa_qn"], np.float32))
    shared["mk_sa"] = head_mask(np.asarray(inputs["sa_kn"], np.float32))

    r64 = np.zeros((D, D), np.float32)
    half = D // 2
    for j in range(half):
        r64[j, j + half] = -1.0  # rot[j] = -x[j+32]
        r64[j + half, j] = 1.0  # rot[j+32] = x[j]
    rt = r64.T  # lhsT (matmul computes lhsT.T @ rhs)
    rotm = np.zeros((P, P), np.float32)
    rotm[:D, :D] = rt
    rotm[D:, D:] = rt
    shared["rotm"] = rotm.astype(BF).copy()

    shared["ones_c"] = np.ones((P, 1), BF)
    shared["ones_r128"] = np.ones((1, P), BF)

    in_maps = []
    for c in range(NCORES):
        s, part = c // NR, c % NR
        rows = slice(part * TOK, (part + 1) * TOK)
        m = dict(shared)
        m["tgtT"] = _featmajor(tgt[s, rows], np.float32)
        m["srcT"] = _featmajor(src[s, rows], F8)
        cq, sq_ = _rope_tables(tgt_pos[s, rows])
        ck, sk = _rope_tables(src_pos[s, rows])
        m["cosq"], m["sinq"] = cq, sq_
        m["coska"], m["sinka"] = ck, sk
        in_maps.append(m)
    return in_maps


def _get_nc():
    if "nc" not in _cache:
        _cache["nc"] = _build_bass()
    return _cache["nc"]


def run(inputs, trace=False):
    """Run on 8 cores; returns (full_output, exec_time_ns_or_None)."""
    if trace:
        _install_ntff_hook()
    from concourse.bass_utils import run_bass_kernel_spmd

    in_maps = _prep_inputs(inputs)
    nc = _get_nc()
    res = run_bass_kernel_spmd(
        nc, in_maps, core_ids=list(range(NCORES)), trace=trace
    )
    out = np.empty((B, N, DIM), np.float32)
    for c in range(NCORES):
        s, part = c // NR, c % NR
        arr = np.asarray(res.results[c]["outT"])  # [128, 8, TOK]
        rows = slice(part * TOK, (part + 1) * TOK)
        out[s, rows] = np.transpose(arr, (2, 1, 0)).reshape(TOK, DIM)
    return out, res.exec_time_ns


def kernel(**inputs):
    out, _ = run(inputs, trace=False)
    return out



# revision 10
# speedup vs baseline: 1.3557x; 1.3557x over previous
"""Trainium2 Bass kernel for nn_CrossLayer (dense transformer layer), v4.

Sharding: sequence-parallel over 8 cores (2 samples x 4 token-chunks of 512).
Each core computes its 512 token rows through CA -> SA -> FFN.  k/v for
both attention blocks are computed from each core's own 512 rows and
AllGather'd across the 4 cores of its sample in 4 pipelined head-group
chunks (fp8 wire), overlapped with attention of earlier groups.

v4 changes vs v3:
- quad layout for q/k: head-quad tiles [128, 2, T] with partition
  p -> head 4g+(p//32), dim d = 32*j + p%32.  Rope's rotate-half becomes
  a free-dim (j) swap: no PE rotation matmul, no ACT rot copy; cos/sin
  tables carry the qn/8 per-channel factors (host-folded).
- rms-norm Ln/Exp batched per quad ([4,T] ACT ops, 4x fewer).
- DoubleRow scores: q/k stored [32, 2, .] per head; contract 64 = 32
  partitions x 2 free-slots; scores run at 0.5 cyc/row like the other
  attention matmuls.
- PSUM rebalanced into two tags: "big" [128,2,512] bufs=3 (6 banks,
  shared by projections/scores/FFN) + "px" (2 banks, AV accumulate).
  Scores double/triple-buffer against the softmax EXP on ACT, which is
  the attention-phase critical engine; the PE no longer ping-pongs with
  ACT.
- attention: fully fp8 e4m3 with DoubleRow matmuls everywhere.
  Weights pre-scaled x8; /8s fold into the cos/sin tables and the
  softmax-denominator exp bias (-ln 64).
- FFN: bf16 end to end (fp8 FFN measured 1.1-1.8e-2 rel err vs the
  2e-2 gate - too risky).
"""

import math
import sys
import types

import numpy as np
import ml_dtypes

B, N, DIM, HID, H, D = 2, 2048, 1024, 4096, 16, 64
TOK = 512  # tokens per core
NCORES = 8
EPS = 1e-6
THETA = 10000.0
P = 128
KO = DIM // P  # 8 contraction chunks
KOP = KO // 2  # 4 DoubleRow pair-chunks
HH = H // 2  # 8 head pairs
HC = HID // P  # 32 hidden chunks
NR = 4  # ranks per replica group
SRCN = 2048  # gathered kv tokens
SKC = SRCN // P  # 16 key chunks of 128 tokens
VW = D + 1  # v columns + ones column
HG = 4  # head groups (quads: 4 heads each)
WS = 8.0  # attention weight pre-scale for fp8
LN64 = math.log(64.0)

BF = ml_dtypes.bfloat16
F8 = ml_dtypes.float8_e4m3

_cache = {}


def _quad_perm():
    """New output-channel order o' = 256*g + 128*j + p for quad layout:
    orig channel c = 64*(4g + p//32) + 32*j + (p%32)."""
    perm = np.empty(DIM, np.int64)
    for g in range(4):
        for j in range(2):
            for p in range(P):
                perm[256 * g + 128 * j + p] = (
                    64 * (4 * g + p // 32) + 32 * j + (p % 32)
                )
    return perm


_QPERM = _quad_perm()


def _lhsT_dr(W):
    """[K, M] -> [M//128, 128(K%128), K//256, 2, 128(M%128)] fp8 x8.
    Slice [mt][:, kp] is a DoubleRow lhsT [128, 2, 128]."""
    K, M = W.shape
    A = W.reshape(K // 256, 2, P, M // P, P)
    return (A.transpose(3, 2, 0, 1, 4) * WS).astype(F8).copy()


def _rhs_dr(W):
    """[K, M] -> [128, K//256, 2, M] fp8 x8 rhs-style DoubleRow moving."""
    K, M = W.shape
    A = W.reshape(K // 256, 2, P, M)
    return (A.transpose(2, 0, 1, 3) * WS).astype(F8).copy()


def _lhsT_bf(W):
    """[K, M] -> [M//128, 128(K%128), K//128, 128(M%128)] bf16 (unscaled)."""
    K, M = W.shape
    return W.reshape(K // P, P, M // P, P).transpose(2, 1, 0, 3).astype(BF).copy()


def _featmajor(x, dt):
    """[tok, dim] -> [128, dim//128, tok]."""
    n = x.shape[0]
    return x.T.reshape(DIM // P, P, n).transpose(1, 0, 2).astype(dt).copy()


def _rope_tables_quad(pos, nv):
    """pos [n] int32, nv [64] norm weights -> (cos2, sinpm) [128, 2, n] bf16.
    cos2[p,j,t] = cos(pos_t * invf[p%32]) * nv[32j + p%32] / 8
    sinpm[p,0,t] = -sin(.) * nv[32 + p%32] / 8   (d0[:,j] = v1[:,1-j]*sinpm[:,j])
    sinpm[p,1,t] = +sin(.) * nv[p%32] / 8
    """
    n = pos.shape[0]
    invf = 1.0 / (THETA ** (np.arange(0, D, 2, dtype=np.float64) / D))  # [32]
    pm32 = np.tile(np.arange(32), 4)  # p % 32 for p in 0..127
    ang = pos.astype(np.float64)[None, :] * invf[pm32][:, None]  # [128, n]
    c = np.cos(ang)
    s = np.sin(ang)
    nv = np.asarray(nv, np.float64)
    cos2 = np.empty((P, 2, n), np.float64)
    sinpm = np.empty((P, 2, n), np.float64)
    cos2[:, 0, :] = c * (nv[pm32] / WS)[:, None]
    cos2[:, 1, :] = c * (nv[32 + pm32] / WS)[:, None]
    sinpm[:, 0, :] = -s * (nv[32 + pm32] / WS)[:, None]
    sinpm[:, 1, :] = s * (nv[pm32] / WS)[:, None]
    return cos2.astype(BF).copy(), sinpm.astype(BF).copy()


def _install_ntff_hook():
    try:
        from trn_agent_boot.trn_boot import _ntff_profile_via_ctypes
    except ImportError:
        return
    if "antenv.axon_hooks" in sys.modules:
        return
    try:
        hook = _ntff_profile_via_ctypes("/opt/axon/libaxon_pjrt.so")
    except OSError:
        return
    mod = types.ModuleType("antenv.axon_hooks")
    mod.get_axon_ntff_profile_hook = lambda: hook
    mod.set_axon_ntff_profile_hook = lambda h: None
    sys.modules["antenv.axon_hooks"] = mod
    import antenv

    antenv.axon_hooks = mod


def _split_multiwait(nc):
    """This walrus only supports one sync-wait on CTRL-encoded instructions
    (Drain/NoOp); hoist excess waits onto single-wait NoOps placed before."""
    from concourse import mybir

    n_split = 0
    for f in nc.m.functions:
        for bb in f.blocks:
            new = []
            changed = False
            for ins in bb.instructions:
                si = ins.sync_info
                if (
                    si is not None
                    and si.on_wait is not None
                    and len(si.on_wait) > 1
                ):
                    waits = list(si.on_wait)
                    keep, rest = waits[:1], waits[1:]
                    for k, w in enumerate(rest):
                        new.append(
                            mybir.InstNoOp(
                                name=f"{ins.name}-wsplit{k}",
                                engine=ins.engine,
                                sync_info=mybir.SyncInfo(
                                    on_wait=[w], on_update=[]
                                ),
                                bass_nofuse=True,
                            )
                        )
                    si.on_wait = keep
                    n_split += 1
                    changed = True
                new.append(ins)
            if changed:
                bb.instructions = new
    return n_split


def _build_bass():
    from contextlib import ExitStack

    import concourse.bass as bass
    import concourse.tile as tile
    from concourse import mybir

    f32 = mybir.dt.float32
    bf16 = mybir.dt.bfloat16
    fp8 = mybir.dt.float8e4
    AF = mybir.ActivationFunctionType
    DR = mybir.MatmulPerfMode.DoubleRow

    nc = bass.Bass(num_devices=NCORES)

    def inp(name, shape, dt=fp8):
        return nc.dram_tensor(name, shape, dt, kind="ExternalInput")

    tgtT = inp("tgtT", [P, KO, TOK], f32)
    srcT = inp("srcT", [P, KO, TOK])
    # rope/norm tables: [128, 2, TOK] bf16 per (pos-set, norm-vec)
    tab_names = ["cqca", "ckca", "cqsa", "cksa"]
    tabs_in = {}
    for tn in tab_names:
        tabs_in[tn] = (
            inp(tn + "_c", [P, 2, TOK], bf16),
            inp(tn + "_s", [P, 2, TOK], bf16),
        )
    caWq = inp("caWq", [HH, P, KOP, 2, P])
    caWk = inp("caWk", [HH, P, KOP, 2, P])
    caWv = inp("caWv", [P, KOP, 2, DIM])
    caWo = inp("caWo", [KO, P, KOP, 2, P])
    saWq = inp("saWq", [HH, P, KOP, 2, P])
    saWk = inp("saWk", [HH, P, KOP, 2, P])
    saWv = inp("saWv", [P, KOP, 2, DIM])
    saWo = inp("saWo", [KO, P, KOP, 2, P])
    W1i = inp("W1", [HC, P, KO, P], bf16)
    W3i = inp("W3", [HC, P, KO, P], bf16)
    W2i = inp("W2", [KO, P, HC, P], bf16)
    blk4 = inp("blk4", [P, 4], bf16)  # per-head ssq lhsT (block ones)
    mask4 = inp("mask4", [4, P], bf16)  # rsqrt bcast lhsT (block ones)
    ones_c = inp("ones_c", [P, 1], bf16)  # y-norm ssq lhsT
    ones_r128 = inp("ones_r128", [1, P], bf16)  # y-norm bcast lhsT

    outT = nc.dram_tensor("outT", [P, KO, TOK], f32, kind="ExternalOutput")

    groups = [[0, 1, 2, 3], [4, 5, 6, 7]]
    KSZ = P * 2 * TOK  # k fp8 words per rank per head-group
    VSZ = P * 4 * 4 * VW  # v fp8 words per rank per head-group

    with tile.TileContext(nc) as tc:
        ctx = ExitStack()
        with ctx:
            sing = ctx.enter_context(tc.tile_pool(name="sing", bufs=1))
            big = ctx.enter_context(tc.tile_pool(name="big", bufs=1))
            wpool = ctx.enter_context(tc.tile_pool(name="wpool", bufs=2))
            w13p = ctx.enter_context(tc.tile_pool(name="w13p", bufs=2))
            w2p = ctx.enter_context(tc.tile_pool(name="w2p", bufs=1))
            htp = ctx.enter_context(tc.tile_pool(name="htp", bufs=2))
            work = ctx.enter_context(tc.tile_pool(name="work", bufs=2))
            probp = ctx.enter_context(tc.tile_pool(name="probp", bufs=2))
            stat = ctx.enter_context(tc.tile_pool(name="stat", bufs=2))
            dram = ctx.enter_context(
                tc.tile_pool(name="dram", bufs=1, space="DRAM")
            )
            # PSUM: "big" [128,2,512] bufs=3 (6 banks) + "px" (2 banks)
            psum = ctx.enter_context(tc.tile_pool(name="psum", bufs=3, space="PSUM"))

            def big_ps(name):
                return psum.tile([P, 2, TOK], f32, tag="big", name=name)

            def small_ps(name, part=P):
                t = psum.tile([part, 2, TOK], f32, tag="big", name=name)
                return t

            # ---- resident tiles
            resid = sing.tile([P, KO, TOK], f32)
            nc.sync.dma_start(resid[:], tgtT[:])
            srcT_sb = sing.tile([P, KO, TOK], fp8, name="srcT_sb")
            nc.sync.dma_start(srcT_sb[:], srcT[:])
            tabs_sb = {}
            for tn in tab_names:
                c_t, s_t = tabs_in[tn]
                cs = sing.tile([P, 2, TOK], bf16, name=tn + "_c")
                nc.sync.dma_start(cs[:], c_t[:])
                ss = sing.tile([P, 2, TOK], bf16, name=tn + "_s")
                nc.sync.dma_start(ss[:], s_t[:])
                tabs_sb[tn] = (cs, ss)
            blk4_sb = sing.tile([P, 4], bf16)
            nc.sync.dma_start(blk4_sb[:], blk4[:])
            mask4_sb = sing.tile([4, P], bf16)
            nc.sync.dma_start(mask4_sb[:], mask4[:])
            ones_c_sb = sing.tile([P, 1], bf16)
            nc.sync.dma_start(ones_c_sb[:], ones_c[:])
            ones_r128_sb = sing.tile([1, P], bf16)
            nc.sync.dma_start(ones_r128_sb[:], ones_r128[:])
            eps_sb = sing.tile([4, 1], f32)
            nc.vector.memset(eps_sb[:], float(EPS))
            bm3_sb = sing.tile([P, 1], f32)
            nc.vector.memset(bm3_sb[:], -3.0)
            bln64_sb = sing.tile([P, 1], f32)
            nc.vector.memset(bln64_sb[:], -LN64)

            yT = sing.tile([P, KO, TOK], fp8, name="yT")
            yF = sing.tile([P, KO, TOK], bf16, name="yF")
            q4 = sing.tile([P, HG, 2, TOK], fp8, name="q4")
            xT = sing.tile([P, HH, TOK], fp8, name="xT")
            k_mine = sing.tile([P, HG, 2, TOK], fp8, name="k_mine")
            v_mine = sing.tile([P, 4, H, VW], fp8, name="v_mine")
            nc.vector.memset(v_mine[:, :, :, D : D + 1], 1.0)
            k_full = big.tile([P, HG, 2, SRCN], fp8, tag="k_full", name="k_full")
            v_full = big.tile([P, SKC, H, VW], fp8, tag="v_full", name="v_full")
            nc.vector.memset(v_full[:, :, :, D : D + 1], 1.0)

            def proj_quad(pq, wq, ysrc):
                """8 DR matmuls: quad projection into pq [128, 2, T]."""
                for j in range(2):
                    for kp in range(KOP):
                        nc.tensor.matmul(
                            pq[:, j, :],
                            wq[:, j, kp],
                            ysrc[:, 2 * kp : 2 * kp + 2, :],
                            start=(kp == 0),
                            stop=(kp == KOP - 1),
                            perf_mode=DR,
                        )

            def norm_rope_quad(pq, tabname, dst):
                """pq PSUM [128(quad), 2, T] f32 at 8x scale -> dst fp8:
                rms-normed, qn-scaled (via tables), roped (j-swap)."""
                cos2_sb, sinpm_sb = tabs_sb[tabname]
                raw = work.tile([P, 2, TOK], bf16, tag="raw", name="raw")
                nc.scalar.copy(raw[:], pq[:])
                sq = work.tile([P, 2, TOK], bf16, tag="sq", name="sq")
                nc.vector.tensor_mul(sq[:], raw[:], raw[:])
                nb = big_ps("nb")  # ssq in bank 0, bc broadcast in bank 1
                ssq = nb[0:4, 0, :]
                for j in range(2):
                    nc.tensor.matmul(
                        ssq,
                        blk4_sb[:],
                        sq[:, j, :],
                        start=(j == 0),
                        stop=(j == 1),
                    )
                # rsqrt(mean+eps) = exp(-0.5*ln(mean+eps)); 1/(64*D) unscales
                # the x8 weight prescale (squared).
                lnt = stat.tile([4, TOK], f32, tag="lnt", name="lnt")
                nc.scalar.activation(
                    lnt[:], ssq, AF.Ln, bias=eps_sb[:], scale=1.0 / (64 * D)
                )
                rs = stat.tile([4, TOK], bf16, tag="rs", name="rs")
                nc.scalar.activation(rs[:], lnt[:], AF.Exp, scale=-0.5)
                bc = nb[:, 1, :]
                nc.tensor.matmul(bc, mask4_sb[:], rs[:], start=True, stop=True)
                v1 = work.tile([P, 2, TOK], bf16, tag="v1", name="v1")
                for j in range(2):
                    nc.vector.tensor_mul(v1[:, j, :], raw[:, j, :], bc)
                t1 = work.tile([P, 2, TOK], bf16, tag="t1", name="t1")
                nc.vector.tensor_mul(t1[:], v1[:], cos2_sb[:])
                d0 = work.tile([P, 2, TOK], bf16, tag="d0", name="d0")
                for j in range(2):
                    nc.vector.tensor_mul(
                        d0[:, j, :], v1[:, 1 - j, :], sinpm_sb[:, j, :]
                    )
                nc.vector.tensor_add(dst, t1[:], d0[:])

            def kv_group_and_ag(g, ysrc, Wk_t, Wv_t, tabname, kv_in, kv_out):
                """k (quad g) + v (4 heads) from my 512 rows -> AG."""
                wk = wpool.tile([P, 2, KOP, 2, P], fp8, tag="wk2", name="wk")
                nc.sync.dma_start(
                    wk[:],
                    Wk_t[2 * g : 2 * g + 2].rearrange("h p a b m -> p h a b m"),
                )
                wv = wpool.tile([P, KOP, 2, 256], fp8, tag="wv", name="wv")
                nc.sync.dma_start(
                    wv[:], Wv_t[:, :, :, g * 256 : (g + 1) * 256]
                )
                for t in range(4):
                    pvt = small_ps("pv")
                    pv = pvt[:, 0, 0:256]
                    for kp in range(KOP):
                        nc.tensor.matmul(
                            pv,
                            ysrc[:, 2 * kp : 2 * kp + 2, t * P : (t + 1) * P],
                            wv[:, kp],
                            start=(kp == 0),
                            stop=(kp == KOP - 1),
                            perf_mode=DR,
                        )
                    nc.vector.tensor_copy(
                        v_mine[:, t, 4 * g : 4 * g + 4, 0:D],
                        pv.rearrange("p (h d) -> p h d", d=D),
                    )
                pq = big_ps("pqk")
                proj_quad(pq, wk, ysrc)
                norm_rope_quad(pq, tabname, k_mine[:, g])
                nc.sync.dma_start(
                    kv_in[:KSZ].rearrange("(p j t) -> p j t", p=P, j=2, t=TOK),
                    k_mine[:, g],
                )
                nc.sync.dma_start(
                    kv_in[KSZ:].rearrange(
                        "(p a b c) -> p a b c", p=P, a=4, b=4, c=VW
                    ),
                    v_mine[:, :, 4 * g : 4 * g + 4, :],
                )
                nc.gpsimd.collective_compute(
                    "AllGather",
                    mybir.AluOpType.bypass,
                    replica_groups=groups,
                    ins=[kv_in.opt()],
                    outs=[kv_out.opt()],
                )

            def scatter_group(g, kv_out):
                for r in range(NR):
                    nc.sync.dma_start(
                        k_full[:, g, :, r * TOK : (r + 1) * TOK],
                        kv_out[r, :KSZ].rearrange(
                            "(p j t) -> p j t", p=P, j=2, t=TOK
                        ),
                    )
                    nc.sync.dma_start(
                        v_full[:, r * 4 : (r + 1) * 4, 4 * g : 4 * g + 4, :],
                        kv_out[r, KSZ:].rearrange(
                            "(p a b c) -> p a b c", p=P, a=4, b=4, c=VW
                        ),
                    )

            def proj_q(Wt, tabname):
                """y -> q (all 4 quads), normed+roped into q4."""
                for g in range(HG):
                    wq = wpool.tile([P, 2, KOP, 2, P], fp8, tag="wk2", name="wq")
                    nc.sync.dma_start(
                        wq[:],
                        Wt[2 * g : 2 * g + 2].rearrange("h p a b m -> p h a b m"),
                    )
                    pq = big_ps("pq")
                    proj_quad(pq, wq, yT)
                    norm_rope_quad(pq, tabname, q4[:, g])

            def attention_group(hg, kdb):
                """scores+softmax+AV for quad hg (pairs j=0,1); fills
                xT[:, 2hg:2hg+2] with x_norm/8 (fp8)."""
                xraw = stat.tile([P, 2, TOK], bf16, tag="xraw", name="xraw")
                dens4 = work.tile(
                    [P, 4, TOK], bf16, tag="dens", bufs=1, name="dens4"
                )
                for j in range(2):
                    px = psum.tile(
                        [VW, 2, TOK], f32, tag="px", bufs=1, name="px"
                    )
                    for kc in range(SKC):
                        ps = big_ps("ps")
                        for i in range(2):
                            b = 2 * j + i
                            nc.tensor.matmul(
                                ps[:, i, :],
                                k_full[
                                    32 * b : 32 * b + 32,
                                    hg,
                                    :,
                                    kc * P : (kc + 1) * P,
                                ],
                                q4[32 * b : 32 * b + 32, hg],
                                start=True,
                                stop=True,
                                perf_mode=DR,
                                tile_position=(32 * b, 0),
                            )
                        if kc % 2 == 0:
                            prob = probp.tile(
                                [P, 2, 2, TOK], fp8, tag="prob", name="prob"
                            )
                        nc.scalar.activation(
                            prob[:, kc % 2],
                            ps[:],
                            AF.Exp,
                            scale=1.0 / math.sqrt(D),
                            bias=bm3_sb[:],
                        )
                        if kc % 2 == 1:
                            j2 = kc - 1
                            for i in range(2):
                                nc.tensor.matmul(
                                    px[:, i, :],
                                    v_full[:, j2 : j2 + 2, hg * 4 + 2 * j + i, :],
                                    prob[:, :, i, :],
                                    start=(kc == 1),
                                    stop=(kc == SKC - 1),
                                    perf_mode=DR,
                                )
                    for i in range(2):
                        # denom row rides on partition 64 (ones column of v)
                        nc.vector.tensor_copy(
                            dens4[D : D + 1, 2 * j + i], px[D : D + 1, i, :]
                        )
                        nc.vector.tensor_copy(
                            xraw[i * D : (i + 1) * D, j], px[0:D, i, :]
                        )
                # reciprocals: 1/(64*den); the 64 unscales v and Wo x8 each,
                # making xT = x_norm/8 which Wo's x8 restores
                nc.sync.dma_start(
                    kdb[: 4 * TOK].rearrange("(o f t) -> o f t", o=1, f=4),
                    dens4[D : D + 1],
                )
                d4 = stat.tile([4, TOK], bf16, tag="d4", name="d4")
                nc.sync.dma_start(
                    d4[:], kdb[: 4 * TOK].rearrange("(f t) -> f t", f=4)
                )
                nc.scalar.activation(d4[:], d4[:], AF.Ln)
                rec4 = stat.tile([4, TOK], bf16, tag="rec4", name="rec4")
                nc.scalar.activation(
                    rec4[:], d4[:], AF.Exp, scale=-1.0, bias=bln64_sb[:4]
                )
                nc.sync.dma_start(
                    kdb[4 * TOK :].rearrange("(f t) -> f t", f=4), rec4[:]
                )
                rec_bc = work.tile([P, 2, TOK], bf16, tag="recbc", name="rec_bc")
                for i in range(2):
                    src = bass.AP(
                        tensor=kdb.tensor,
                        offset=kdb.offset + 4 * TOK + i * TOK,
                        ap=[[0, D], [2 * TOK, 2], [1, TOK]],
                    )
                    nc.sync.dma_start(rec_bc[i * D : (i + 1) * D], src)
                for j in range(2):
                    nc.vector.tensor_mul(
                        xT[:, 2 * hg + j], xraw[:, j], rec_bc[:, j]
                    )

            def wo_group(hg, Wo_t):
                """Wo partial for head-quad hg, accumulated into resid."""
                wo = wpool.tile([P, KO, 2, P], fp8, tag="wo", name="wo")
                nc.sync.dma_start(
                    wo[:], Wo_t[:, :, hg].rearrange("o p b m -> p o b m")
                )
                for oc in range(KO):
                    pot = small_ps("po")
                    po = pot[:, 0, :]
                    nc.tensor.matmul(
                        po,
                        wo[:, oc],
                        xT[:, 2 * hg : 2 * hg + 2, :],
                        start=True,
                        stop=True,
                        perf_mode=DR,
                    )
                    nc.vector.tensor_add(resid[:, oc], resid[:, oc], po)

            def rmsnorm_feat(dst):
                """resid f32 -> dst (fp8 or bf16): resid * rsqrt(mean sq)."""
                nb = big_ps("ynb")  # ssq in bank 0, bc broadcast in bank 1
                ssq = nb[0:1, 0, :]
                for c in range(KO):
                    sq = work.tile([P, TOK], bf16, tag="ysq", name="ynsq")
                    nc.vector.tensor_mul(sq[:], resid[:, c], resid[:, c])
                    nc.tensor.matmul(
                        ssq,
                        ones_c_sb[:],
                        sq[:],
                        start=(c == 0),
                        stop=(c == KO - 1),
                    )
                lnt = stat.tile([1, TOK], f32, tag="lnt", name="ylnt")
                nc.scalar.activation(
                    lnt[:], ssq, AF.Ln, bias=eps_sb[:1], scale=1.0 / DIM
                )
                rs = stat.tile([1, TOK], bf16, tag="rs", name="yrs")
                nc.scalar.activation(rs[:], lnt[:], AF.Exp, scale=-0.5)
                bc = nb[:, 1, :]
                nc.tensor.matmul(bc, ones_r128_sb[:], rs[:], start=True, stop=True)
                for c in range(KO):
                    nc.vector.tensor_mul(dst[:, c], resid[:, c], bc)

            # ================= cross-attention =================
            # CA kv depends only on src: fire projections + AllGathers first
            # so they overlap the collectives entry barrier.
            kvi_ca = [
                dram.tile([KSZ + VSZ], fp8, tag=f"kvica{g}", name=f"kvica{g}")
                for g in range(HG)
            ]
            kvo_ca = [
                dram.tile([NR, KSZ + VSZ], fp8, tag=f"kvoca{g}", name=f"kvoca{g}")
                for g in range(HG)
            ]
            kdbs = [
                dram.tile([8 * TOK], bf16, tag=f"kdb{g}", name=f"kdb{g}")
                for g in range(HG)
            ]
            for g in range(HG):
                kv_group_and_ag(
                    g, srcT_sb, caWk, caWv, "ckca", kvi_ca[g], kvo_ca[g]
                )
            rmsnorm_feat(yT)
            proj_q(caWq, "cqca")
            for hg in range(HG):
                scatter_group(hg, kvo_ca[hg])
                attention_group(hg, kdbs[hg])
                if hg >= 1:
                    wo_group(hg - 1, caWo)
            wo_group(HG - 1, caWo)

            # ================= self-attention =================
            rmsnorm_feat(yT)
            kvi_sa = [
                dram.tile([KSZ + VSZ], fp8, tag=f"kvisa{g}", name=f"kvisa{g}")
                for g in range(HG)
            ]
            kvo_sa = [
                dram.tile([NR, KSZ + VSZ], fp8, tag=f"kvosa{g}", name=f"kvosa{g}")
                for g in range(HG)
            ]
            kdbs2 = [
                dram.tile([8 * TOK], bf16, tag=f"kdc{g}", name=f"kdc{g}")
                for g in range(HG)
            ]
            for g in range(HG):
                kv_group_and_ag(
                    g, yT, saWk, saWv, "cksa", kvi_sa[g], kvo_sa[g]
                )
                if g == 0:
                    proj_q(saWq, "cqsa")
            for hg in range(HG):
                scatter_group(hg, kvo_sa[hg])
                attention_group(hg, kdbs2[hg])
                if hg >= 1:
                    wo_group(hg - 1, saWo)
            wo_group(HG - 1, saWo)

            # ================= FFN (bf16) =================
            rmsnorm_feat(yF)
            for qtr in range(4):
                hT = htp.tile([P, 8, TOK], bf16, tag="hT", bufs=1, name="hT")
                for e in range(2):
                    w1 = w13p.tile([P, 4, KO, P], bf16, tag="w13", name="w1")
                    nc.sync.dma_start(
                        w1[:],
                        W1i[qtr * 8 + e * 4 : qtr * 8 + e * 4 + 4].rearrange(
                            "h p a m -> p h a m"
                        ),
                    )
                    w3 = w13p.tile([P, 4, KO, P], bf16, tag="w13", name="w3")
                    nc.sync.dma_start(
                        w3[:],
                        W3i[qtr * 8 + e * 4 : qtr * 8 + e * 4 + 4].rearrange(
                            "h p a m -> p h a m"
                        ),
                    )
                    for gg in range(4):
                        p13 = big_ps("p13")
                        p1 = p13[:, 0, :]
                        p3 = p13[:, 1, :]
                        for c in range(KO):
                            nc.tensor.matmul(
                                p1, w1[:, gg, c], yF[:, c],
                                start=(c == 0), stop=(c == KO - 1),
                            )
                        for c in range(KO):
                            nc.tensor.matmul(
                                p3, w3[:, gg, c], yF[:, c],
                                start=(c == 0), stop=(c == KO - 1),
                            )
                        s1 = stat.tile([P, TOK], bf16, tag="s1", name="s1")
                        nc.scalar.activation(s1[:], p1, AF.Silu)
                        nc.vector.tensor_mul(hT[:, e * 4 + gg], s1[:], p3)
                w2 = w2p.tile([P, KO, 8, P], bf16, tag="w2", name="w2")
                nc.sync.dma_start(
                    w2[:],
                    W2i[:, :, qtr * 8 : (qtr + 1) * 8].rearrange(
                        "o p a m -> p o a m"
                    ),
                )
                for oc in range(KO):
                    pot = small_ps("po2")
                    po = pot[:, 0, :]
                    for gg in range(8):
                        nc.tensor.matmul(
                            po, w2[:, oc, gg], hT[:, gg],
                            start=(gg == 0), stop=(gg == 7),
                        )
                    nc.vector.tensor_add(resid[:, oc], resid[:, oc], po)

            nc.sync.dma_start(outT[:], resid[:])

    _split_multiwait(nc)
    return nc


def _prep_inputs(inputs):
    """Full problem inputs -> list of 8 per-core in_maps."""
    tgt = np.asarray(inputs["tgt"], np.float32)
    src = np.asarray(inputs["src"], np.float32)
    tgt_pos = np.asarray(inputs["tgt_pos"], np.int32)
    src_pos = np.asarray(inputs["src_pos"], np.int32)

    pre_ca_w = np.asarray(inputs["pre_ca_w"], np.float32)
    pre_sa_w = np.asarray(inputs["pre_sa_w"], np.float32)
    pre_ffn_w = np.asarray(inputs["pre_ffn_w"], np.float32)

    def fold(Wname, w):
        return np.asarray(inputs[Wname], np.float32) * w[:, None]

    ca_Wq = fold("ca_Wq", pre_ca_w)
    ca_Wkv = np.asarray(inputs["ca_Wkv"], np.float32)
    ca_Wk, ca_Wv = ca_Wkv[:, :DIM], ca_Wkv[:, DIM:]
    ca_Wo = np.asarray(inputs["ca_Wo"], np.float32)
    sa_Wq = fold("sa_Wq", pre_sa_w)
    sa_Wkv = fold("sa_Wkv", pre_sa_w)
    sa_Wk, sa_Wv = sa_Wkv[:, :DIM], sa_Wkv[:, DIM:]
    sa_Wo = np.asarray(inputs["sa_Wo"], np.float32)
    W1 = fold("W1", pre_ffn_w)
    W3 = fold("W3", pre_ffn_w)
    W2 = np.asarray(inputs["W2"], np.float32)

    shared = {
        "caWq": _lhsT_dr(ca_Wq[:, _QPERM]),
        "caWk": _lhsT_dr(ca_Wk[:, _QPERM]),
        "caWv": _rhs_dr(ca_Wv),
        "caWo": _lhsT_dr(ca_Wo),
        "saWq": _lhsT_dr(sa_Wq[:, _QPERM]),
        "saWk": _lhsT_dr(sa_Wk[:, _QPERM]),
        "saWv": _rhs_dr(sa_Wv),
        "saWo": _lhsT_dr(sa_Wo),
        "W1": _lhsT_bf(W1),
        "W3": _lhsT_bf(W3),
        "W2": _lhsT_bf(W2),
    }

    blk4 = np.zeros((P, 4), np.float32)
    for m in range(4):
        blk4[32 * m : 32 * m + 32, m] = 1
    shared["blk4"] = blk4.astype(BF).copy()
    shared["mask4"] = blk4.T.astype(BF).copy()
    shared["ones_c"] = np.ones((P, 1), BF)
    shared["ones_r128"] = np.ones((1, P), BF)

    ca_qn = np.asarray(inputs["ca_qn"], np.float32)
    ca_kn = np.asarray(inputs["ca_kn"], np.float32)
    sa_qn = np.asarray(inputs["sa_qn"], np.float32)
    sa_kn = np.asarray(inputs["sa_kn"], np.float32)

    in_maps = []
    for c in range(NCORES):
        s, part = c // NR, c % NR
        rows = slice(part * TOK, (part + 1) * TOK)
        m = dict(shared)
        m["tgtT"] = _featmajor(tgt[s, rows], np.float32)
        m["srcT"] = _featmajor(src[s, rows], F8)
        tpos = tgt_pos[s, rows]
        spos = src_pos[s, rows]
        for tn, (pos, nv) in {
            "cqca": (tpos, ca_qn),
            "ckca": (spos, ca_kn),
            "cqsa": (tpos, sa_qn),
            "cksa": (tpos, sa_kn),
        }.items():
            ct, st = _rope_tables_quad(pos, nv)
            m[tn + "_c"] = ct
            m[tn + "_s"] = st
        in_maps.append(m)
    return in_maps


def _get_nc():
    if "nc" not in _cache:
        _cache["nc"] = _build_bass()
    return _cache["nc"]


def run(inputs, trace=False):
    """Run on 8 cores; returns (full_output, exec_time_ns_or_None)."""
    if trace:
        _install_ntff_hook()
    from concourse.bass_utils import run_bass_kernel_spmd

    in_maps = _prep_inputs(inputs)
    nc = _get_nc()
    res = run_bass_kernel_spmd(
        nc, in_maps, core_ids=list(range(NCORES)), trace=trace
    )
    out = np.empty((B, N, DIM), np.float32)
    for c in range(NCORES):
        s, part = c // NR, c % NR
        arr = np.asarray(res.results[c]["outT"])  # [128, 8, TOK]
        rows = slice(part * TOK, (part + 1) * TOK)
        out[s, rows] = np.transpose(arr, (2, 1, 0)).reshape(TOK, DIM)
    return out, res.exec_time_ns


def kernel(**inputs):
    out, _ = run(inputs, trace=False)
    return out


# revision 13
# speedup vs baseline: 1.3661x; 1.0077x over previous
"""Trainium2 Bass kernel for nn_CrossLayer (dense transformer layer), v4.

Sharding: sequence-parallel over 8 cores (2 samples x 4 token-chunks of 512).
Each core computes its 512 token rows through CA -> SA -> FFN.  k/v for
both attention blocks are computed from each core's own 512 rows and
AllGather'd across the 4 cores of its sample in 4 pipelined head-group
chunks (fp8 wire), overlapped with attention of earlier groups.

v4 changes vs v3:
- quad layout for q/k: head-quad tiles [128, 2, T] with partition
  p -> head 4g+(p//32), dim d = 32*j + p%32.  Rope's rotate-half becomes
  a free-dim (j) swap: no PE rotation matmul, no ACT rot copy; cos/sin
  tables carry the qn/8 per-channel factors (host-folded).
- rms-norm Ln/Exp batched per quad ([4,T] ACT ops, 4x fewer).
- DoubleRow scores: q/k stored [32, 2, .] per head; contract 64 = 32
  partitions x 2 free-slots; scores run at 0.5 cyc/row like the other
  attention matmuls.
- PSUM rebalanced into two tags: "big" [128,2,512] bufs=3 (6 banks,
  shared by projections/scores/FFN) + "px" (2 banks, AV accumulate).
  Scores double/triple-buffer against the softmax EXP on ACT, which is
  the attention-phase critical engine; the PE no longer ping-pongs with
  ACT.
- attention: fully fp8 e4m3 with DoubleRow matmuls everywhere.
  Weights pre-scaled x8; /8s fold into the cos/sin tables and the
  softmax-denominator exp bias (-ln 64).
- FFN: bf16 end to end (fp8 FFN measured 1.1-1.8e-2 rel err vs the
  2e-2 gate - too risky).
"""

import math
import sys
import types

import numpy as np
import ml_dtypes

B, N, DIM, HID, H, D = 2, 2048, 1024, 4096, 16, 64
TOK = 512  # tokens per core
NCORES = 8
EPS = 1e-6
THETA = 10000.0
P = 128
KO = DIM // P  # 8 contraction chunks
KOP = KO // 2  # 4 DoubleRow pair-chunks
HH = H // 2  # 8 head pairs
HC = HID // P  # 32 hidden chunks
NR = 4  # ranks per replica group
SRCN = 2048  # gathered kv tokens
SKC = SRCN // P  # 16 key chunks of 128 tokens
VW = D + 1  # v columns + ones column
HG = 4  # head groups (quads: 4 heads each)
WS = 8.0  # attention weight pre-scale for fp8
LN64 = math.log(64.0)

BF = ml_dtypes.bfloat16
F8 = ml_dtypes.float8_e4m3

_cache = {}


def _quad_perm():
    """New output-channel order o' = 256*g + 128*j + p for quad layout:
    orig channel c = 64*(4g + p//32) + 32*j + (p%32)."""
    perm = np.empty(DIM, np.int64)
    for g in range(4):
        for j in range(2):
            for p in range(P):
                perm[256 * g + 128 * j + p] = (
                    64 * (4 * g + p // 32) + 32 * j + (p % 32)
                )
    return perm


_QPERM = _quad_perm()


def _lhsT_dr(W):
    """[K, M] -> [M//128, 128(K%128), K//256, 2, 128(M%128)] fp8 x8.
    Slice [mt][:, kp] is a DoubleRow lhsT [128, 2, 128]."""
    K, M = W.shape
    A = W.reshape(K // 256, 2, P, M // P, P)
    return (A.transpose(3, 2, 0, 1, 4) * WS).astype(F8).copy()


def _rhs_dr(W):
    """[K, M] -> [128, K//256, 2, M] fp8 x8 rhs-style DoubleRow moving."""
    K, M = W.shape
    A = W.reshape(K // 256, 2, P, M)
    return (A.transpose(2, 0, 1, 3) * WS).astype(F8).copy()


def _lhsT_bf(W):
    """[K, M] -> [M//128, 128(K%128), K//128, 128(M%128)] bf16 (unscaled)."""
    K, M = W.shape
    return W.reshape(K // P, P, M // P, P).transpose(2, 1, 0, 3).astype(BF).copy()


def _featmajor(x, dt):
    """[tok, dim] -> [128, dim//128, tok]."""
    n = x.shape[0]
    return x.T.reshape(DIM // P, P, n).transpose(1, 0, 2).astype(dt).copy()


def _rope_tables_quad(pos, nv):
    """pos [n] int32, nv [64] norm weights -> (cos2, sinpm) [128, 2, n] bf16.
    cos2[p,j,t] = cos(pos_t * invf[p%32]) * nv[32j + p%32] / 8
    sinpm[p,0,t] = -sin(.) * nv[32 + p%32] / 8   (d0[:,j] = v1[:,1-j]*sinpm[:,j])
    sinpm[p,1,t] = +sin(.) * nv[p%32] / 8
    """
    n = pos.shape[0]
    invf = 1.0 / (THETA ** (np.arange(0, D, 2, dtype=np.float64) / D))  # [32]
    pm32 = np.tile(np.arange(32), 4)  # p % 32 for p in 0..127
    ang = pos.astype(np.float64)[None, :] * invf[pm32][:, None]  # [128, n]
    c = np.cos(ang)
    s = np.sin(ang)
    nv = np.asarray(nv, np.float64)
    cos2 = np.empty((P, 2, n), np.float64)
    sinpm = np.empty((P, 2, n), np.float64)
    cos2[:, 0, :] = c * (nv[pm32] / WS)[:, None]
    cos2[:, 1, :] = c * (nv[32 + pm32] / WS)[:, None]
    sinpm[:, 0, :] = -s * (nv[32 + pm32] / WS)[:, None]
    sinpm[:, 1, :] = s * (nv[pm32] / WS)[:, None]
    return cos2.astype(BF).copy(), sinpm.astype(BF).copy()


def _install_ntff_hook():
    try:
        from trn_agent_boot.trn_boot import _ntff_profile_via_ctypes
    except ImportError:
        return
    if "antenv.axon_hooks" in sys.modules:
        return
    try:
        hook = _ntff_profile_via_ctypes("/opt/axon/libaxon_pjrt.so")
    except OSError:
        return
    mod = types.ModuleType("antenv.axon_hooks")
    mod.get_axon_ntff_profile_hook = lambda: hook
    mod.set_axon_ntff_profile_hook = lambda h: None
    sys.modules["antenv.axon_hooks"] = mod
    import antenv

    antenv.axon_hooks = mod


def _split_multiwait(nc):
    """This walrus only supports one sync-wait on CTRL-encoded instructions
    (Drain/NoOp); hoist excess waits onto single-wait NoOps placed before."""
    from concourse import mybir

    n_split = 0
    for f in nc.m.functions:
        for bb in f.blocks:
            new = []
            changed = False
            for ins in bb.instructions:
                si = ins.sync_info
                if (
                    si is not None
                    and si.on_wait is not None
                    and len(si.on_wait) > 1
                ):
                    waits = list(si.on_wait)
                    keep, rest = waits[:1], waits[1:]
                    for k, w in enumerate(rest):
                        new.append(
                            mybir.InstNoOp(
                                name=f"{ins.name}-wsplit{k}",
                                engine=ins.engine,
                                sync_info=mybir.SyncInfo(
                                    on_wait=[w], on_update=[]
                                ),
                                bass_nofuse=True,
                            )
                        )
                    si.on_wait = keep
                    n_split += 1
                    changed = True
                new.append(ins)
            if changed:
                bb.instructions = new
    return n_split


def _build_bass():
    from contextlib import ExitStack

    import concourse.bass as bass
    import concourse.tile as tile
    from concourse import mybir

    f32 = mybir.dt.float32
    bf16 = mybir.dt.bfloat16
    fp8 = mybir.dt.float8e4
    AF = mybir.ActivationFunctionType
    DR = mybir.MatmulPerfMode.DoubleRow

    nc = bass.Bass(num_devices=NCORES)

    def inp(name, shape, dt=fp8):
        return nc.dram_tensor(name, shape, dt, kind="ExternalInput")

    tgtT = inp("tgtT", [P, KO, TOK], f32)
    srcT = inp("srcT", [P, KO, TOK])
    # rope/norm tables: [128, 2, TOK] bf16 per (pos-set, norm-vec)
    tab_names = ["cqca", "ckca", "cqsa", "cksa"]
    tabs_in = {}
    for tn in tab_names:
        tabs_in[tn] = (
            inp(tn + "_c", [P, 2, TOK], bf16),
            inp(tn + "_s", [P, 2, TOK], bf16),
        )
    caWq = inp("caWq", [HH, P, KOP, 2, P])
    caWk = inp("caWk", [HH, P, KOP, 2, P])
    caWv = inp("caWv", [P, KOP, 2, DIM])
    caWo = inp("caWo", [KO, P, KOP, 2, P])
    saWq = inp("saWq", [HH, P, KOP, 2, P])
    saWk = inp("saWk", [HH, P, KOP, 2, P])
    saWv = inp("saWv", [P, KOP, 2, DIM])
    saWo = inp("saWo", [KO, P, KOP, 2, P])
    W1i = inp("W1", [HC, P, KO, P], bf16)
    W3i = inp("W3", [HC, P, KO, P], bf16)
    W2i = inp("W2", [KO, P, HC, P], bf16)
    blk4 = inp("blk4", [P, 4], bf16)  # per-head ssq lhsT (block ones)
    mask4 = inp("mask4", [4, P], bf16)  # rsqrt bcast lhsT (block ones)
    ones_c = inp("ones_c", [P, 1], bf16)  # y-norm ssq lhsT
    ones_r128 = inp("ones_r128", [1, P], bf16)  # y-norm bcast lhsT

    outT = nc.dram_tensor("outT", [P, KO, TOK], f32, kind="ExternalOutput")

    groups = [[0, 1, 2, 3], [4, 5, 6, 7]]
    KSZ = P * 2 * TOK  # k fp8 words per rank per head-group
    VSZ = P * 4 * 4 * VW  # v fp8 words per rank per head-group

    with tile.TileContext(nc) as tc:
        ctx = ExitStack()
        with ctx:
            sing = ctx.enter_context(tc.tile_pool(name="sing", bufs=1))
            big = ctx.enter_context(tc.tile_pool(name="big", bufs=1))
            wpool = ctx.enter_context(tc.tile_pool(name="wpool", bufs=2))
            w13p = ctx.enter_context(tc.tile_pool(name="w13p", bufs=2))
            w2p = ctx.enter_context(tc.tile_pool(name="w2p", bufs=1))
            htp = ctx.enter_context(tc.tile_pool(name="htp", bufs=2))
            work = ctx.enter_context(tc.tile_pool(name="work", bufs=2))
            probp = ctx.enter_context(tc.tile_pool(name="probp", bufs=2))
            stat = ctx.enter_context(tc.tile_pool(name="stat", bufs=2))
            dram = ctx.enter_context(
                tc.tile_pool(name="dram", bufs=1, space="DRAM")
            )
            # PSUM: "big" [128,2,512] bufs=3 (6 banks) + "px" (2 banks)
            psum = ctx.enter_context(tc.tile_pool(name="psum", bufs=3, space="PSUM"))

            def big_ps(name):
                return psum.tile([P, 2, TOK], f32, tag="big", name=name)

            def small_ps(name, part=P):
                t = psum.tile([part, 2, TOK], f32, tag="big", name=name)
                return t

            # ---- warmup collective: soak CC-core init + core skew before
            # the first real AllGather
            wu_in = dram.tile([256], fp8, tag="wu_in", name="wu_in")
            wu_out = dram.tile([NR, 256], fp8, tag="wu_out", name="wu_out")
            nc.gpsimd.collective_compute(
                "AllGather",
                mybir.AluOpType.bypass,
                replica_groups=groups,
                ins=[wu_in.opt()],
                outs=[wu_out.opt()],
            )

            # ---- resident tiles (DMA order: CA-kv critical path first)
            srcT_sb = sing.tile([P, KO, TOK], fp8, name="srcT_sb")
            nc.sync.dma_start(srcT_sb[:], srcT[:])
            blk4_sb = sing.tile([P, 4], bf16)
            nc.sync.dma_start(blk4_sb[:], blk4[:])
            mask4_sb = sing.tile([4, P], bf16)
            nc.sync.dma_start(mask4_sb[:], mask4[:])
            tabs_sb = {}

            def load_tab(tn):
                c_t, s_t = tabs_in[tn]
                cs = sing.tile([P, 2, TOK], bf16, name=tn + "_c")
                nc.sync.dma_start(cs[:], c_t[:])
                ss = sing.tile([P, 2, TOK], bf16, name=tn + "_s")
                nc.sync.dma_start(ss[:], s_t[:])
                tabs_sb[tn] = (cs, ss)

            load_tab("ckca")
            eps_sb = sing.tile([4, 1], f32)
            nc.vector.memset(eps_sb[:], float(EPS))
            bm3_sb = sing.tile([P, 1], f32)
            nc.vector.memset(bm3_sb[:], -3.0)
            bln64_sb = sing.tile([P, 1], f32)
            nc.vector.memset(bln64_sb[:], -LN64)
            resid = sing.tile([P, KO, TOK], f32)
            ones_c_sb = sing.tile([P, 1], bf16)
            ones_r128_sb = sing.tile([1, P], bf16)

            yT = sing.tile([P, KO, TOK], fp8, name="yT")
            yF = sing.tile([P, KO, TOK], bf16, name="yF")
            q4 = sing.tile([P, HG, 2, TOK], fp8, name="q4")
            xT = sing.tile([P, HH, TOK], fp8, name="xT")
            k_mine = sing.tile([P, HG, 2, TOK], fp8, name="k_mine")
            v_mine = sing.tile([P, 4, H, VW], fp8, name="v_mine")
            nc.vector.memset(v_mine[:, :, :, D : D + 1], 1.0)
            k_full = big.tile([P, HG, 2, SRCN], fp8, tag="k_full", name="k_full")
            v_full = big.tile([P, SKC, H, VW], fp8, tag="v_full", name="v_full")
            nc.vector.memset(v_full[:, :, :, D : D + 1], 1.0)

            def proj_quad(pq, wq, ysrc):
                """8 DR matmuls: quad projection into pq [128, 2, T]."""
                for j in range(2):
                    for kp in range(KOP):
                        nc.tensor.matmul(
                            pq[:, j, :],
                            wq[:, j, kp],
                            ysrc[:, 2 * kp : 2 * kp + 2, :],
                            start=(kp == 0),
                            stop=(kp == KOP - 1),
                            perf_mode=DR,
                        )

            def norm_rope_quad(pq, tabname, dst):
                """pq PSUM [128(quad), 2, T] f32 at 8x scale -> dst fp8:
                rms-normed, qn-scaled (via tables), roped (j-swap)."""
                cos2_sb, sinpm_sb = tabs_sb[tabname]
                raw = work.tile([P, 2, TOK], bf16, tag="raw", name="raw")
                nc.scalar.copy(raw[:], pq[:])
                sq = work.tile([P, 2, TOK], bf16, tag="sq", name="sq")
                nc.vector.tensor_mul(sq[:], raw[:], raw[:])
                nb = big_ps("nb")  # ssq in bank 0, bc broadcast in bank 1
                ssq = nb[0:4, 0, :]
                for j in range(2):
                    nc.tensor.matmul(
                        ssq,
                        blk4_sb[:],
                        sq[:, j, :],
                        start=(j == 0),
                        stop=(j == 1),
                    )
                # rsqrt(mean+eps) = exp(-0.5*ln(mean+eps)); 1/(64*D) unscales
                # the x8 weight prescale (squared).
                lnt = stat.tile([4, TOK], f32, tag="lnt", name="lnt")
                nc.scalar.activation(
                    lnt[:], ssq, AF.Ln, bias=eps_sb[:], scale=1.0 / (64 * D)
                )
                rs = stat.tile([4, TOK], bf16, tag="rs", name="rs")
                nc.scalar.activation(rs[:], lnt[:], AF.Exp, scale=-0.5)
                bc = nb[:, 1, :]
                nc.tensor.matmul(bc, mask4_sb[:], rs[:], start=True, stop=True)
                v1 = work.tile([P, 2, TOK], bf16, tag="v1", name="v1")
                for j in range(2):
                    nc.vector.tensor_mul(v1[:, j, :], raw[:, j, :], bc)
                t1 = work.tile([P, 2, TOK], bf16, tag="t1", name="t1")
                nc.vector.tensor_mul(t1[:], v1[:], cos2_sb[:])
                d0 = work.tile([P, 2, TOK], bf16, tag="d0", name="d0")
                for j in range(2):
                    nc.vector.tensor_mul(
                        d0[:, j, :], v1[:, 1 - j, :], sinpm_sb[:, j, :]
                    )
                nc.vector.tensor_add(dst, t1[:], d0[:])

            def kv_group_and_ag(g, ysrc, Wk_t, Wv_t, tabname, kv_in, kv_out):
                """k (quad g) + v (4 heads) from my 512 rows -> AG."""
                wk = wpool.tile([P, 2, KOP, 2, P], fp8, tag="wk2", name="wk")
                nc.sync.dma_start(
                    wk[:],
                    Wk_t[2 * g : 2 * g + 2].rearrange("h p a b m -> p h a b m"),
                )
                wv = wpool.tile([P, KOP, 2, 256], fp8, tag="wv", name="wv")
                nc.sync.dma_start(
                    wv[:], Wv_t[:, :, :, g * 256 : (g + 1) * 256]
                )
                for t in range(4):
                    pvt = small_ps("pv")
                    pv = pvt[:, 0, 0:256]
                    for kp in range(KOP):
                        nc.tensor.matmul(
                            pv,
                            ysrc[:, 2 * kp : 2 * kp + 2, t * P : (t + 1) * P],
                            wv[:, kp],
                            start=(kp == 0),
                            stop=(kp == KOP - 1),
                            perf_mode=DR,
                        )
                    nc.vector.tensor_copy(
                        v_mine[:, t, 4 * g : 4 * g + 4, 0:D],
                        pv.rearrange("p (h d) -> p h d", d=D),
                    )
                pq = big_ps("pqk")
                proj_quad(pq, wk, ysrc)
                norm_rope_quad(pq, tabname, k_mine[:, g])
                nc.sync.dma_start(
                    kv_in[:KSZ].rearrange("(p j t) -> p j t", p=P, j=2, t=TOK),
                    k_mine[:, g],
                )
                nc.sync.dma_start(
                    kv_in[KSZ:].rearrange(
                        "(p a b c) -> p a b c", p=P, a=4, b=4, c=VW
                    ),
                    v_mine[:, :, 4 * g : 4 * g + 4, :],
                )
                nc.gpsimd.collective_compute(
                    "AllGather",
                    mybir.AluOpType.bypass,
                    replica_groups=groups,
                    ins=[kv_in.opt()],
                    outs=[kv_out.opt()],
                )

            def scatter_group(g, kv_out):
                for r in range(NR):
                    nc.sync.dma_start(
                        k_full[:, g, :, r * TOK : (r + 1) * TOK],
                        kv_out[r, :KSZ].rearrange(
                            "(p j t) -> p j t", p=P, j=2, t=TOK
                        ),
                    )
                    nc.sync.dma_start(
                        v_full[:, r * 4 : (r + 1) * 4, 4 * g : 4 * g + 4, :],
                        kv_out[r, KSZ:].rearrange(
                            "(p a b c) -> p a b c", p=P, a=4, b=4, c=VW
                        ),
                    )

            def proj_q(Wt, tabname):
                """y -> q (all 4 quads), normed+roped into q4."""
                for g in range(HG):
                    wq = wpool.tile([P, 2, KOP, 2, P], fp8, tag="wk2", name="wq")
                    nc.sync.dma_start(
                        wq[:],
                        Wt[2 * g : 2 * g + 2].rearrange("h p a b m -> p h a b m"),
                    )
                    pq = big_ps("pq")
                    proj_quad(pq, wq, yT)
                    norm_rope_quad(pq, tabname, q4[:, g])

            def attention_group(hg, kdb):
                """scores+softmax+AV for quad hg (pairs j=0,1); fills
                xT[:, 2hg:2hg+2] with x_norm/8 (fp8)."""
                xraw = stat.tile([P, 2, TOK], bf16, tag="xraw", name="xraw")
                dens4 = work.tile(
                    [P, 4, TOK], bf16, tag="dens", bufs=1, name="dens4"
                )
                for j in range(2):
                    px = psum.tile(
                        [VW, 2, TOK], f32, tag="px", bufs=1, name="px"
                    )
                    for kc in range(SKC):
                        ps = big_ps("ps")
                        for i in range(2):
                            b = 2 * j + i
                            nc.tensor.matmul(
                                ps[:, i, :],
                                k_full[
                                    32 * b : 32 * b + 32,
                                    hg,
                                    :,
                                    kc * P : (kc + 1) * P,
                                ],
                                q4[32 * b : 32 * b + 32, hg],
                                start=True,
                                stop=True,
                                perf_mode=DR,
                                tile_position=(32 * b, 0),
                            )
                        if kc % 2 == 0:
                            prob = probp.tile(
                                [P, 2, 2, TOK], fp8, tag="prob", name="prob"
                            )
                        nc.scalar.activation(
                            prob[:, kc % 2],
                            ps[:],
                            AF.Exp,
                            scale=1.0 / math.sqrt(D),
                            bias=bm3_sb[:],
                        )
                        if kc % 2 == 1:
                            j2 = kc - 1
                            for i in range(2):
                                nc.tensor.matmul(
                                    px[:, i, :],
                                    v_full[:, j2 : j2 + 2, hg * 4 + 2 * j + i, :],
                                    prob[:, :, i, :],
                                    start=(kc == 1),
                                    stop=(kc == SKC - 1),
                                    perf_mode=DR,
                                )
                    for i in range(2):
                        # denom row rides on partition 64 (ones column of v)
                        nc.vector.tensor_copy(
                            dens4[D : D + 1, 2 * j + i], px[D : D + 1, i, :]
                        )
                        nc.vector.tensor_copy(
                            xraw[i * D : (i + 1) * D, j], px[0:D, i, :]
                        )
                # reciprocals: 1/(64*den); the 64 unscales v and Wo x8 each,
                # making xT = x_norm/8 which Wo's x8 restores
                nc.sync.dma_start(
                    kdb[: 4 * TOK].rearrange("(o f t) -> o f t", o=1, f=4),
                    dens4[D : D + 1],
                )
                d4 = stat.tile([4, TOK], bf16, tag="d4", name="d4")
                nc.sync.dma_start(
                    d4[:], kdb[: 4 * TOK].rearrange("(f t) -> f t", f=4)
                )
                nc.scalar.activation(d4[:], d4[:], AF.Ln)
                rec4 = stat.tile([4, TOK], bf16, tag="rec4", name="rec4")
                nc.scalar.activation(
                    rec4[:], d4[:], AF.Exp, scale=-1.0, bias=bln64_sb[:4]
                )
                nc.sync.dma_start(
                    kdb[4 * TOK :].rearrange("(f t) -> f t", f=4), rec4[:]
                )
                rec_bc = work.tile([P, 2, TOK], bf16, tag="recbc", name="rec_bc")
                for i in range(2):
                    src = bass.AP(
                        tensor=kdb.tensor,
                        offset=kdb.offset + 4 * TOK + i * TOK,
                        ap=[[0, D], [2 * TOK, 2], [1, TOK]],
                    )
                    nc.sync.dma_start(rec_bc[i * D : (i + 1) * D], src)
                for j in range(2):
                    nc.vector.tensor_mul(
                        xT[:, 2 * hg + j], xraw[:, j], rec_bc[:, j]
                    )

            def wo_group(hg, Wo_t):
                """Wo partial for head-quad hg, accumulated into resid."""
                wo = wpool.tile([P, KO, 2, P], fp8, tag="wo", name="wo")
                nc.sync.dma_start(
                    wo[:], Wo_t[:, :, hg].rearrange("o p b m -> p o b m")
                )
                for oc in range(KO):
                    pot = small_ps("po")
                    po = pot[:, 0, :]
                    nc.tensor.matmul(
                        po,
                        wo[:, oc],
                        xT[:, 2 * hg : 2 * hg + 2, :],
                        start=True,
                        stop=True,
                        perf_mode=DR,
                    )
                    nc.vector.tensor_add(resid[:, oc], resid[:, oc], po)

            def rmsnorm_feat(dst):
                """resid f32 -> dst (fp8 or bf16): resid * rsqrt(mean sq)."""
                nb = big_ps("ynb")  # ssq in bank 0, bc broadcast in bank 1
                ssq = nb[0:1, 0, :]
                for c in range(KO):
                    sq = work.tile([P, TOK], bf16, tag="ysq", name="ynsq")
                    nc.vector.tensor_mul(sq[:], resid[:, c], resid[:, c])
                    nc.tensor.matmul(
                        ssq,
                        ones_c_sb[:],
                        sq[:],
                        start=(c == 0),
                        stop=(c == KO - 1),
                    )
                lnt = stat.tile([1, TOK], f32, tag="lnt", name="ylnt")
                nc.scalar.activation(
                    lnt[:], ssq, AF.Ln, bias=eps_sb[:1], scale=1.0 / DIM
                )
                rs = stat.tile([1, TOK], bf16, tag="rs", name="yrs")
                nc.scalar.activation(rs[:], lnt[:], AF.Exp, scale=-0.5)
                bc = nb[:, 1, :]
                nc.tensor.matmul(bc, ones_r128_sb[:], rs[:], start=True, stop=True)
                for c in range(KO):
                    nc.vector.tensor_mul(dst[:, c], resid[:, c], bc)

            # ================= cross-attention =================
            # CA kv depends only on src: fire projections + AllGathers first
            # so they overlap the collectives entry barrier.
            kvi_ca = [
                dram.tile([KSZ + VSZ], fp8, tag=f"kvica{g}", name=f"kvica{g}")
                for g in range(HG)
            ]
            kvo_ca = [
                dram.tile([NR, KSZ + VSZ], fp8, tag=f"kvoca{g}", name=f"kvoca{g}")
                for g in range(HG)
            ]
            kdbs = [
                dram.tile([8 * TOK], bf16, tag=f"kdb{g}", name=f"kdb{g}")
                for g in range(HG)
            ]
            for g in range(HG):
                kv_group_and_ag(
                    g, srcT_sb, caWk, caWv, "ckca", kvi_ca[g], kvo_ca[g]
                )
                if g == 0:
                    # non-critical resident loads, after the CA-kv DMAs
                    nc.sync.dma_start(resid[:], tgtT[:])
                    nc.sync.dma_start(ones_c_sb[:], ones_c[:])
                    nc.sync.dma_start(ones_r128_sb[:], ones_r128[:])
                    load_tab("cqca")
                elif g == 1:
                    load_tab("cqsa")
                    load_tab("cksa")
            rmsnorm_feat(yT)
            proj_q(caWq, "cqca")
            for hg in range(HG):
                scatter_group(hg, kvo_ca[hg])
                attention_group(hg, kdbs[hg])
                if hg >= 1:
                    wo_group(hg - 1, caWo)
            wo_group(HG - 1, caWo)

            # ================= self-attention =================
            rmsnorm_feat(yT)
            kvi_sa = [
                dram.tile([KSZ + VSZ], fp8, tag=f"kvisa{g}", name=f"kvisa{g}")
                for g in range(HG)
            ]
            kvo_sa = [
                dram.tile([NR, KSZ + VSZ], fp8, tag=f"kvosa{g}", name=f"kvosa{g}")
                for g in range(HG)
            ]
            kdbs2 = [
                dram.tile([8 * TOK], bf16, tag=f"kdc{g}", name=f"kdc{g}")
                for g in range(HG)
            ]
            for g in range(HG):
                kv_group_and_ag(
                    g, yT, saWk, saWv, "cksa", kvi_sa[g], kvo_sa[g]
                )
                if g == 0:
                    proj_q(saWq, "cqsa")
            for hg in range(HG):
                scatter_group(hg, kvo_sa[hg])
                attention_group(hg, kdbs2[hg])
                if hg >= 1:
                    wo_group(hg - 1, saWo)
            wo_group(HG - 1, saWo)

            # ================= FFN (bf16) =================
            rmsnorm_feat(yF)
            for qtr in range(4):
                hT = htp.tile([P, 8, TOK], bf16, tag="hT", name="hT")
                for e in range(4):
                    w1 = w13p.tile([P, 2, KO, P], bf16, tag="w1", name="w1")
                    nc.sync.dma_start(
                        w1[:],
                        W1i[qtr * 8 + e * 2 : qtr * 8 + e * 2 + 2].rearrange(
                            "h p a m -> p h a m"
                        ),
                    )
                    w3 = w13p.tile([P, 2, KO, P], bf16, tag="w3", name="w3")
                    nc.sync.dma_start(
                        w3[:],
                        W3i[qtr * 8 + e * 2 : qtr * 8 + e * 2 + 2].rearrange(
                            "h p a m -> p h a m"
                        ),
                    )
                    for gg in range(2):
                        p13 = big_ps("p13")
                        p1 = p13[:, 0, :]
                        p3 = p13[:, 1, :]
                        for c in range(KO):
                            nc.tensor.matmul(
                                p1, w1[:, gg, c], yF[:, c],
                                start=(c == 0), stop=(c == KO - 1),
                            )
                        for c in range(KO):
                            nc.tensor.matmul(
                                p3, w3[:, gg, c], yF[:, c],
                                start=(c == 0), stop=(c == KO - 1),
                            )
                        s1 = stat.tile([P, TOK], bf16, tag="s1", name="s1")
                        nc.scalar.activation(s1[:], p1, AF.Silu)
                        nc.vector.tensor_mul(hT[:, e * 2 + gg], s1[:], p3)
                w2 = w2p.tile([P, KO, 8, P], bf16, tag="w2", name="w2")
                nc.sync.dma_start(
                    w2[:],
                    W2i[:, :, qtr * 8 : (qtr + 1) * 8].rearrange(
                        "o p a m -> p o a m"
                    ),
                )
                for oc in range(KO):
                    pot = small_ps("po2")
                    po = pot[:, 0, :]
                    for gg in range(8):
                        nc.tensor.matmul(
                            po, w2[:, oc, gg], hT[:, gg],
                            start=(gg == 0), stop=(gg == 7),
                        )
                    nc.vector.tensor_add(resid[:, oc], resid[:, oc], po)

            nc.sync.dma_start(outT[:], resid[:])

    _split_multiwait(nc)
    return nc


def _prep_inputs(inputs):
    """Full problem inputs -> list of 8 per-core in_maps."""
    tgt = np.asarray(inputs["tgt"], np.float32)
    src = np.asarray(inputs["src"], np.float32)
    tgt_pos = np.asarray(inputs["tgt_pos"], np.int32)
    src_pos = np.asarray(inputs["src_pos"], np.int32)

    pre_ca_w = np.asarray(inputs["pre_ca_w"], np.float32)
    pre_sa_w = np.asarray(inputs["pre_sa_w"], np.float32)
    pre_ffn_w = np.asarray(inputs["pre_ffn_w"], np.float32)

    def fold(Wname, w):
        return np.asarray(inputs[Wname], np.float32) * w[:, None]

    ca_Wq = fold("ca_Wq", pre_ca_w)
    ca_Wkv = np.asarray(inputs["ca_Wkv"], np.float32)
    ca_Wk, ca_Wv = ca_Wkv[:, :DIM], ca_Wkv[:, DIM:]
    ca_Wo = np.asarray(inputs["ca_Wo"], np.float32)
    sa_Wq = fold("sa_Wq", pre_sa_w)
    sa_Wkv = fold("sa_Wkv", pre_sa_w)
    sa_Wk, sa_Wv = sa_Wkv[:, :DIM], sa_Wkv[:, DIM:]
    sa_Wo = np.asarray(inputs["sa_Wo"], np.float32)
    W1 = fold("W1", pre_ffn_w)
    W3 = fold("W3", pre_ffn_w)
    W2 = np.asarray(inputs["W2"], np.float32)

    shared = {
        "caWq": _lhsT_dr(ca_Wq[:, _QPERM]),
        "caWk": _lhsT_dr(ca_Wk[:, _QPERM]),
        "caWv": _rhs_dr(ca_Wv),
        "caWo": _lhsT_dr(ca_Wo),
        "saWq": _lhsT_dr(sa_Wq[:, _QPERM]),
        "saWk": _lhsT_dr(sa_Wk[:, _QPERM]),
        "saWv": _rhs_dr(sa_Wv),
        "saWo": _lhsT_dr(sa_Wo),
        "W1": _lhsT_bf(W1),
        "W3": _lhsT_bf(W3),
        "W2": _lhsT_bf(W2),
    }

    blk4 = np.zeros((P, 4), np.float32)
    for m in range(4):
        blk4[32 * m : 32 * m + 32, m] = 1
    shared["blk4"] = blk4.astype(BF).copy()
    shared["mask4"] = blk4.T.astype(BF).copy()
    shared["ones_c"] = np.ones((P, 1), BF)
    shared["ones_r128"] = np.ones((1, P), BF)

    ca_qn = np.asarray(inputs["ca_qn"], np.float32)
    ca_kn = np.asarray(inputs["ca_kn"], np.float32)
    sa_qn = np.asarray(inputs["sa_qn"], np.float32)
    sa_kn = np.asarray(inputs["sa_kn"], np.float32)

    in_maps = []
    for c in range(NCORES):
        s, part = c // NR, c % NR
        rows = slice(part * TOK, (part + 1) * TOK)
        m = dict(shared)
        m["tgtT"] = _featmajor(tgt[s, rows], np.float32)
        m["srcT"] = _featmajor(src[s, rows], F8)
        tpos = tgt_pos[s, rows]
        spos = src_pos[s, rows]
        for tn, (pos, nv) in {
            "cqca": (tpos, ca_qn),
            "ckca": (spos, ca_kn),
            "cqsa": (tpos, sa_qn),
            "cksa": (tpos, sa_kn),
        }.items():
            ct, st = _rope_tables_quad(pos, nv)
            m[tn + "_c"] = ct
            m[tn + "_s"] = st
        in_maps.append(m)
    return in_maps


def _get_nc():
    if "nc" not in _cache:
        _cache["nc"] = _build_bass()
    return _cache["nc"]


def run(inputs, trace=False):
    """Run on 8 cores; returns (full_output, exec_time_ns_or_None)."""
    if trace:
        _install_ntff_hook()
    from concourse.bass_utils import run_bass_kernel_spmd

    in_maps = _prep_inputs(inputs)
    nc = _get_nc()
    res = run_bass_kernel_spmd(
        nc, in_maps, core_ids=list(range(NCORES)), trace=trace
    )
    out = np.empty((B, N, DIM), np.float32)
    for c in range(NCORES):
        s, part = c // NR, c % NR
        arr = np.asarray(res.results[c]["outT"])  # [128, 8, TOK]
        rows = slice(part * TOK, (part + 1) * TOK)
        out[s, rows] = np.transpose(arr, (2, 1, 0)).reshape(TOK, DIM)
    return out, res.exec_time_ns


def kernel(**inputs):
    out, _ = run(inputs, trace=False)
    return out
